# revision 11
# baseline (speedup 1.0000x reference)
"""HSIViT forward on 8 Trainium2 NeuronCores.

Sharding: pure data parallel — batch B=8, one batch item per core, no
collectives. Each core runs the full 12-layer ViT on its (512, 768)
token activations and emits its (100,) logits row.

Host-side prep (numpy, not counted in HW exec time):
  - patch cubes extracted + transposed per batch item (xpT [512, 512])
  - all weights transposed to [c_in, c_out] for the PE's lhsT layout
  - LN1/LN2 scale+bias folded into q/k/v and fc1 weights+biases
  - v weights+bias pre-scaled by SV so the fp8 eviction needs no extra op
  - final feature-LN scale/bias folded into the classifier head
  - weights cast to bf16; patch embed + head ride bf16 too

Schedule (vs the previous revision):
  - qk projection fused into the attention pipeline per output-column
    tile mc: scores for head pair mc follow qkproj(mc+1), so the ACT
    exp stream starts early and stays hidden behind PE work.
  - score matmuls row-packed: both heads of a pair run concurrently in
    disjoint PE row groups (K=64 each), into the two banks of a
    [128,1024] psum tile; one ACT exp covers both heads' j-chunk.
  - exp output is fp8(e4m3); AV runs fp8 DoubleRow (K=256/pass) with a
    ones-column in V producing the softmax denominator; reciprocal
    batched per head pair; normalization fused into the oT eviction.
  - fc2 (and patch) iterate t in (3,0,1,2) so the last token tile's
    LN chain overlaps the other tiles' matmuls; next layer's aT
    transposes then run stall-free.
  - final head weights prefetched in 2 big DMAs; the feature transpose
    runs inline with fc2 of layer 11; a dummy sqrt warms the ACT table.
"""

import os
import sys

import numpy as np

for _p in ("/opt/trn_rl_repo", "/root/.axon_site/_ro/trn_rl_repo"):
    if _p not in sys.path and os.path.isdir(_p):
        sys.path.insert(0, _p)

import ml_dtypes  # noqa: E402

import concourse.bass as bass  # noqa: E402,F401
import concourse.mybir as mybir  # noqa: E402
import concourse.tile as tile  # noqa: E402
from concourse import bacc  # noqa: E402
from concourse.bass_utils import run_bass_kernel_spmd  # noqa: E402
from concourse.masks import make_identity  # noqa: E402

F32 = mybir.dt.float32
BF16 = mybir.dt.bfloat16
FP8 = mybir.dt.float8e4
AF = mybir.ActivationFunctionType
OP = mybir.AluOpType
AX = mybir.AxisListType
DR = mybir.MatmulPerfMode.DoubleRow

DEPTH, C, NH, HD = 12, 768, 12, 64
NTOK, PVEC = 512, 512
FF = 3072
NCLS = 100
TB, SP = 8, 64
FD = TB * C
SCALE = HD**-0.5
EPS = 1e-5
SV = 32.0  # fp8 scale on the v path (weights+bias pre-scaled host-side)
VSL = NH * (HD + 1) + 4  # 784: per-key-pair-slot v row, padded so 784%16==0

CB_PER_LAYER = 36  # qb 6 + kb 6 + f1b 24 columns
CB_W1 = DEPTH * CB_PER_LAYER
CB_HB = CB_W1 + 1
CB_COLS = CB_HB + 1

bf16 = ml_dtypes.bfloat16


def _build():
    nc = bacc.Bacc(None, target_bir_lowering=False, debug=False)

    d_xpt = nc.dram_tensor("xpt", [PVEC, NTOK], BF16, kind="ExternalInput")
    d_pos2 = nc.dram_tensor("pos2", [NTOK, C], F32, kind="ExternalInput")
    d_pwt = nc.dram_tensor("pwt", [PVEC, C], BF16, kind="ExternalInput")
    d_wq = nc.dram_tensor("wq", [DEPTH, C, C], BF16, kind="ExternalInput")
    d_wk = nc.dram_tensor("wk", [DEPTH, C, C], BF16, kind="ExternalInput")
    d_wv = nc.dram_tensor("wv", [DEPTH, C, C], BF16, kind="ExternalInput")
    d_wp = nc.dram_tensor("wp", [DEPTH, C, C], BF16, kind="ExternalInput")
    d_w1 = nc.dram_tensor("w1", [DEPTH, C, FF], BF16, kind="ExternalInput")
    d_w2 = nc.dram_tensor("w2", [DEPTH, FF, C], BF16, kind="ExternalInput")
    d_cb = nc.dram_tensor("cb", [128, CB_COLS], F32, kind="ExternalInput")
    d_rb = nc.dram_tensor("rb", [DEPTH, 3, C], BF16, kind="ExternalInput")
    d_hwt = nc.dram_tensor("hwt", [4, 128, 12 * NCLS], BF16, kind="ExternalInput")
    d_out = nc.dram_tensor("out", [NCLS], F32, kind="ExternalOutput")

    from contextlib import ExitStack

    with tile.TileContext(nc) as tc:
        with ExitStack() as ctx:
            ep = ctx.enter_context
            const = ep(tc.tile_pool(name="const", bufs=1))
            hpool = ep(tc.tile_pool(name="hpool", bufs=4))
            arow_p = ep(tc.tile_pool(name="arow", bufs=4))
            aT_p = ep(tc.tile_pool(name="atp", bufs=6))
            a2T_p = ep(tc.tile_pool(name="a2tp", bufs=6))
            qT_p = ep(tc.tile_pool(name="qtp", bufs=6))
            kT_p = ep(tc.tile_pool(name="ktp", bufs=6))
            vx_p = ep(tc.tile_pool(name="vxp", bufs=2))
            ex_p = ep(tc.tile_pool(name="exp", bufs=3))
            oT_p = ep(tc.tile_pool(name="otp", bufs=6))
            gT_p = ep(tc.tile_pool(name="gtp", bufs=24))
            qkw_p = ep(tc.tile_pool(name="qkw", bufs=12))
            vpw_p = ep(tc.tile_pool(name="vpw", bufs=12))
            w1_p = ep(tc.tile_pool(name="w1p", bufs=12))
            w2_p = ep(tc.tile_pool(name="w2p", bufs=26))
            hw_p = ep(tc.tile_pool(name="hwp", bufs=2))
            bc_p = ep(tc.tile_pool(name="bcp", bufs=2))
            rcp_p = ep(tc.tile_pool(name="rcpp", bufs=2))
            den_p = ep(tc.tile_pool(name="denp", bufs=1))
            sm_p = ep(tc.tile_pool(name="smp", bufs=8))
            sm512_p = ep(tc.tile_pool(name="sm512", bufs=2))
            mm_ps = ep(tc.tile_pool(name="mmps", bufs=3, space="PSUM"))
            sc_ps = ep(tc.tile_pool(name="scps", bufs=2, space="PSUM"))
            tp_ps = ep(tc.tile_pool(name="tpps", bufs=1, space="PSUM"))

            ident = const.tile([128, 128], F32, tag="ident", name="ident")
            make_identity(nc, ident)
            identB = const.tile([128, 128], BF16, tag="identB", name="identB")
            nc.scalar.copy(identB[:], ident[:])
            ones0 = const.tile([128, 1], F32, tag="ones0", name="ones0")
            nc.vector.memset(ones0[:], 1.0)
            onesB = const.tile([128, 1], BF16, tag="onesB", name="onesB")
            nc.scalar.copy(onesB[:], ones0[:])
            eps = const.tile([128, 1], F32, tag="eps", name="eps")
            nc.vector.memset(eps[:], EPS)
            cb = const.tile([128, CB_COLS], F32, tag="cb", name="cb")
            nc.sync.dma_start(out=cb[:], in_=d_cb[:])

            h = []
            for t in range(4):
                ht = hpool.tile([128, C], F32, tag="h", name=f"h{t}")
                h.append(ht)

            def emit_stats0(t, tag):
                st6 = sm_p.tile([128, 12], F32, tag="st6", name=f"st6_{tag}{t}")
                nc.vector.bn_stats(st6[:, 0:6], h[t][:, 0:384])
                return st6

            def emit_ln_rest(t, tag, st6):
                nc.vector.bn_stats(st6[:, 6:12], h[t][:, 384:768])
                mv = sm_p.tile([128, 2], F32, tag="mv", name=f"mv{tag}{t}")
                nc.vector.bn_aggr(mv[:], st6.rearrange("p (g s) -> p g s", g=2))
                std = sm_p.tile([128, 1], F32, tag="std", name=f"std{tag}{t}")
                nc.scalar.activation(std[:], mv[:, 1:2], AF.Sqrt, bias=eps[:])
                rstd = sm_p.tile([128, 1], F32, tag="rstd", name=f"rstd{tag}{t}")
                nc.vector.reciprocal_approx_fast(out=rstd[:], in_=std[:])
                at = arow_p.tile([128, C], BF16, tag="ar", name=f"ar{tag}{t}")
                nc.vector.tensor_scalar(
                    at[:], h[t], mv[:, 0:1], rstd[:], op0=OP.subtract, op1=OP.mult
                )
                return at

            def transpose_pass(rows, t_list, col0, outs, tag2):
                """Transpose token tiles t_list into cols [col0:] of the 6
                col tiles; evictions alternate ACT/DVE on tp-tile halves."""
                w = 128 * len(t_list)
                tp = tp_ps.tile([128, 1024], BF16, tag="tp", name=f"tp{tag2}")
                for cc in range(6):
                    sl = tp[:, (cc % 2) * 512 : (cc % 2) * 512 + 512]
                    for ti, t in enumerate(t_list):
                        nc.tensor.transpose(
                            sl[:, ti * 128 : (ti + 1) * 128],
                            rows[t][:, cc * 128 : (cc + 1) * 128],
                            identB[:],
                        )
                    dst = outs[cc][:, col0 : col0 + w]
                    if cc % 2 == 0:
                        nc.scalar.copy(dst, sl[:, 0:w])
                    else:
                        nc.vector.tensor_copy(dst, sl[:, 0:w])

            def bcast_row(i, j, tag):
                src = sm512_p.tile([1, C], BF16, tag="rbs", name=f"rbs{i}_{j}", bufs=1)
                nc.sync.dma_start(out=src[:], in_=d_rb[i, j])
                bt = bc_p.tile([128, C], BF16, tag="bc", name=f"{tag}{i}")
                nc.gpsimd.partition_broadcast(bt[:], src[:])
                return bt

            # ---- patch embed: h = pos(+patch_b) + xp @ patch_w.T ----
            xpt = []
            pwt = []
            for kc in range(4):
                xt = aT_p.tile([128, NTOK], BF16, tag="at", name=f"xpt{kc}")
                nc.sync.dma_start(out=xt[:], in_=d_xpt[kc * 128 : (kc + 1) * 128, :])
                xpt.append(xt)
            for kc in range(4):
                wt = vpw_p.tile([128, C], BF16, tag="vpw", name=f"pwt{kc}")
                nc.sync.dma_start(out=wt[:], in_=d_pwt[kc * 128 : (kc + 1) * 128, :])
                pwt.append(wt)
            a_rows = [None] * 4
            for t in (3, 0, 1, 2):
                nc.sync.dma_start(out=h[t][:], in_=d_pos2[t * 128 : (t + 1) * 128, :])
                st6 = None
                for n in range(2):
                    ns = slice(n * 384, (n + 1) * 384)
                    ps = mm_ps.tile([128, 512], F32, tag="mm", name=f"pep{t}{n}")
                    for kc in range(4):
                        nc.tensor.matmul(
                            ps[:, :384],
                            xpt[kc][:, t * 128 : (t + 1) * 128],
                            pwt[kc][:, ns],
                            start=(kc == 0),
                            stop=(kc == 3),
                        )
                    nc.vector.tensor_tensor(h[t][:, ns], h[t][:, ns], ps[:, :384], op=OP.add)
                    if n == 0:
                        st6 = emit_stats0(t, "a0_")
                a_rows[t] = emit_ln_rest(t, "a0_", st6)

            for i in range(DEPTH):
                cb0 = i * CB_PER_LAYER
                # ---- transpose all 4 LN1 token tiles ----
                aT = [
                    aT_p.tile([128, NTOK], BF16, tag="at", name=f"at{cc}")
                    for cc in range(6)
                ]
                transpose_pass(a_rows, (0, 1, 2, 3), 0, aT, f"at{i}_")

                # ---- weights for this layer ----
                qk_w = []
                for (dw, tg) in ((d_wq, "qw"), (d_wk, "kw")):
                    wts = []
                    for kc in range(6):
                        wt = qkw_p.tile([128, C], BF16, tag="qkw", name=f"{tg}{kc}")
                        nc.sync.dma_start(out=wt[:], in_=dw[i, kc * 128 : (kc + 1) * 128, :])
                        wts.append(wt)
                    qk_w.append(wts)
                vwts = []
                for kc in range(6):
                    wt = vpw_p.tile([128, C], BF16, tag="vpw", name=f"vw{kc}")
                    nc.sync.dma_start(out=wt[:], in_=d_wv[i, kc * 128 : (kc + 1) * 128, :])
                    vwts.append(wt)
                pwts = []
                for kc in range(6):
                    wt = vpw_p.tile([128, C], BF16, tag="vpw", name=f"pw{kc}")
                    nc.sync.dma_start(out=wt[:], in_=d_wp[i, kc * 128 : (kc + 1) * 128, :])
                    pwts.append(wt)

                vbB = bcast_row(i, 0, "vb")
                pbB = bcast_row(i, 1, "pb")

                # v extended tiles: [key, slot(2), 12*(64+1)+pad] fp8, slot =
                # key-chunk within the DoubleRow pair; ones col at 64 of each
                # 65-wide head block gives the softmax denominator.
                vx = []
                for g in range(2):
                    vt = vx_p.tile([128, 2 * VSL], FP8, tag="vx", name=f"vx{g}")
                    vx.append(vt)
                    for s in range(2):
                        ones_sl = vt[:, s * VSL : s * VSL + NH * 65].rearrange(
                            "p (h d) -> p h d", h=NH
                        )[:, :, HD : HD + 1]
                        nc.vector.memset(ones_sl, 1.0)

                qT = [None] * 6
                kT = [None] * 6
                ex_all = [None] * 6
                po_all = [None] * NH
                oT = []
                for cc in range(6):
                    ot = oT_p.tile([128, NTOK], BF16, tag="ot", name=f"ot{cc}")
                    oT.append(ot)

                def emit_qkproj(mc):
                    for (wts, outs, base) in ((qk_w[0], qT, cb0), (qk_w[1], kT, cb0 + 6)):
                        ps = mm_ps.tile([128, 512], F32, tag="mm", name=f"qkp{mc}")
                        for k in range(6):
                            kc = (k + mc) % 6
                            nc.tensor.matmul(
                                ps[:],
                                wts[kc][:, mc * 128 : (mc + 1) * 128],
                                aT[kc][:],
                                start=(k == 0),
                                stop=(k == 5),
                            )
                        out = (qT_p if outs is qT else kT_p).tile(
                            [128, NTOK], BF16, tag="qt" if outs is qT else "kt",
                            name=f"{'q' if outs is qT else 'k'}T{mc}",
                        )
                        nc.vector.tensor_scalar_add(
                            out[:], ps[:], cb[:, base + mc : base + mc + 1]
                        )
                        outs[mc] = out

                def emit_pair_scores(p):
                    # heads 2p (rows 0:64) and 2p+1 (rows 64:128) of qT/kT[p],
                    # row-packed into the two banks of a [128,1024] psum tile;
                    # exp -> fp8 ex tile laid out [j(4), hE|hO (2x512)].
                    expair = ex_p.tile([128, 4096], FP8, tag="ex", name=f"ex{p}")
                    for j in range(4):
                        sc = sc_ps.tile([128, 1024], F32, tag="sc", name=f"sc{p}_{j}")
                        for hi in range(2):
                            off = hi * 64
                            nc.tensor.matmul(
                                sc[:, hi * 512 : (hi + 1) * 512],
                                kT[p][off : off + 64, j * 128 : (j + 1) * 128],
                                qT[p][off : off + 64, :],
                                start=True,
                                stop=True,
                            )
                        nc.scalar.activation(
                            expair[:, j * 1024 : (j + 1) * 1024], sc[:], AF.Exp,
                            scale=SCALE,
                        )
                    ex_all[p] = expair

                def emit_vgroup(t):
                    vxt = vx[t // 2]
                    base = (t % 2) * VSL
                    for n in range(2):
                        ps = mm_ps.tile([128, 512], F32, tag="mm", name=f"vp{t}{n}")
                        for kc in range(6):
                            nc.tensor.matmul(
                                ps[:, :384],
                                aT[kc][:, t * 128 : (t + 1) * 128],
                                vwts[kc][:, n * 384 : (n + 1) * 384],
                                start=(kc == 0),
                                stop=(kc == 5),
                            )
                        dst = vxt[:, base + n * 6 * 65 : base + (n + 1) * 6 * 65].rearrange(
                            "p (h d) -> p h d", h=6
                        )[:, :, 0:HD]
                        nc.vector.tensor_tensor(
                            dst,
                            ps[:, :384].rearrange("p (g d) -> p g d", g=6),
                            vbB[:, n * 384 : (n + 1) * 384].rearrange(
                                "p (g d) -> p g d", g=6
                            ),
                            op=OP.add,
                        )

                def emit_pair_av(p):
                    exr = ex_all[p].rearrange("q (j c) -> q j c", j=4)
                    for hi in range(2):
                        hh = 2 * p + hi
                        po = mm_ps.tile([128, 512], F32, tag="mm", name=f"po{hh}")
                        for jp in range(2):
                            nc.tensor.matmul(
                                po[0 : HD + 1, :],
                                vx[jp].rearrange("q (s c) -> q s c", s=2)[
                                    :, :, hh * 65 : hh * 65 + 65
                                ],
                                exr[:, 2 * jp : 2 * jp + 2, hi * 512 : (hi + 1) * 512],
                                start=(jp == 0),
                                stop=(jp == 1),
                                perf_mode=DR,
                            )
                        den = den_p.tile([1, NTOK], F32, tag="dr", name=f"den{hh}", bufs=2)
                        nc.vector.tensor_scalar_mul(den[:], po[HD : HD + 1, :], SV)
                        rcp = den_p.tile([1, NTOK], F32, tag="dr", name=f"rcp{hh}", bufs=2)
                        nc.vector.reciprocal_approx_fast(out=rcp[:], in_=den[:])
                        rb_ = rcp_p.tile([64, NTOK], F32, tag="rb", name=f"rcpB{hh}")
                        nc.gpsimd.partition_broadcast(rb_[:], rcp[:])
                        nc.vector.tensor_tensor(
                            oT[hh // 2][(hh % 2) * 64 : (hh % 2) * 64 + 64, :],
                            po[0:HD, :],
                            rb_[:],
                            op=OP.mult,
                        )

                # pipeline: qkproj leads scores by 1 tile; vgroups early; AV
                # lags scores by 2 pairs so exps drain on ACT.
                for mc in range(6):
                    emit_qkproj(mc)
                    if mc >= 1:
                        emit_pair_scores(mc - 1)
                    if mc == 1:
                        emit_vgroup(0)
                        emit_vgroup(1)
                    if mc == 2:
                        emit_vgroup(2)
                        emit_vgroup(3)
                        nc.vector.tensor_tensor(h[0][:], h[0][:], pbB[:], op=OP.add)
                        nc.vector.tensor_tensor(h[1][:], h[1][:], pbB[:], op=OP.add)
                    if mc == 3:
                        nc.vector.tensor_tensor(h[2][:], h[2][:], pbB[:], op=OP.add)
                        nc.vector.tensor_tensor(h[3][:], h[3][:], pbB[:], op=OP.add)
                    if mc >= 3:
                        emit_pair_av(mc - 3)
                emit_pair_scores(5)
                emit_pair_av(3)
                emit_pair_av(4)
                emit_pair_av(5)

                # ---- output projection + residual (pb pre-added), LN2 inline ----
                a2_rows = [None] * 4
                for t in range(4):
                    st6 = None
                    for n in range(2):
                        ns = slice(n * 384, (n + 1) * 384)
                        ps = mm_ps.tile([128, 512], F32, tag="mm", name=f"prj{t}{n}")
                        for k in range(6):
                            kc = (k + 1) % 6
                            nc.tensor.matmul(
                                ps[:, :384],
                                oT[kc][:, t * 128 : (t + 1) * 128],
                                pwts[kc][:, ns],
                                start=(k == 0),
                                stop=(k == 5),
                            )
                        nc.vector.tensor_tensor(h[t][:, ns], h[t][:, ns], ps[:, :384], op=OP.add)
                        if n == 0:
                            st6 = emit_stats0(t, f"b{i}_")
                    a2_rows[t] = emit_ln_rest(t, f"b{i}_", st6)
                a2T = [
                    a2T_p.tile([128, NTOK], BF16, tag="a2t", name=f"a2t{cc}")
                    for cc in range(6)
                ]
                transpose_pass(a2_rows, (0, 1, 2), 0, a2T, f"a2t{i}_")

                # ---- fc1 + gelu; first half's 0:384 pass hides t3's LN+transpose ----
                f2bB = bcast_row(i, 2, "fb")
                gT = [gT_p.tile([128, NTOK], BF16, tag="gt", name=f"gt{m}") for m in range(24)]
                for half in range(2):
                    wts = []
                    for kc in range(6):
                        wt = w1_p.tile([128, FF // 2], BF16, tag="w1", name=f"w1_{half}_{kc}")
                        nc.sync.dma_start(
                            out=wt[:],
                            in_=d_w1[
                                i,
                                kc * 128 : (kc + 1) * 128,
                                half * (FF // 2) : (half + 1) * (FF // 2),
                            ],
                        )
                        wts.append(wt)
                    for cs, ce in ((0, 384), (384, 512)):
                        if cs == 384 and half == 0:
                            transpose_pass(a2_rows, (3,), 384, a2T, f"a2u{i}_")
                        w = ce - cs
                        for mh in range(12):
                            m = half * 12 + mh
                            ps = mm_ps.tile([128, 512], F32, tag="mm", name=f"f1p{m}")
                            for k in range(6):
                                kc = (k + mh) % 6
                                nc.tensor.matmul(
                                    ps[:, 0:w],
                                    wts[kc][:, mh * 128 : (mh + 1) * 128],
                                    a2T[kc][:, cs:ce],
                                    start=(k == 0),
                                    stop=(k == 5),
                                )
                            nc.scalar.activation(
                                gT[m][:, cs:ce], ps[:, 0:w], AF.Gelu,
                                bias=cb[:, cb0 + 12 + m : cb0 + 13 + m],
                            )
                        if half == 0 and cs == 0:
                            for t in range(4):
                                nc.vector.tensor_tensor(h[t][:], h[t][:], f2bB[:], op=OP.add)

                # ---- fc2 + residual; t order (3,0,1,2); next LN1 (or final
                # feature transpose for the last layer) inline ----
                last = i == DEPTH - 1
                if last:
                    # prefetch head weights; warm the sqrt table while ACT idles
                    hw = []
                    for g in range(4):
                        hwt_t = hw_p.tile([128, 12 * NCLS], BF16, tag="hw", name=f"hw{g}")
                        nc.sync.dma_start(out=hwt_t[:], in_=d_hwt[g])
                        hw.append(hwt_t)
                    dum = sm_p.tile([1, 1], F32, tag="dum", name="dum")
                    nc.scalar.activation(dum[:], eps[0:1, :], AF.Sqrt)
                    hTa = w1_p.tile([128, 3 * NTOK], BF16, tag="w1", name="hTa")
                    hTb = w1_p.tile([128, 3 * NTOK], BF16, tag="w1", name="hTb")

                    def hTr(cc):
                        t_ = hTa if cc < 3 else hTb
                        return t_.rearrange("p (c w) -> p c w", c=3)[:, cc % 3, :]

                    def emit_ftr(t, hbt):
                        tp = tp_ps.tile([128, 1024], BF16, tag="tp", name=f"tpf{t}")
                        for cc in range(6):
                            nc.tensor.transpose(
                                tp[:, cc * 128 : (cc + 1) * 128],
                                hbt[:, cc * 128 : (cc + 1) * 128],
                                identB[:],
                            )
                        srcs = tp[:, 0:768].rearrange("p (c w) -> p c w", c=6)
                        for gg, dtile in ((0, hTa), (1, hTb)):
                            dst = dtile.rearrange("p (c w) -> p c w", c=3)[
                                :, :, t * 128 : (t + 1) * 128
                            ]
                            if (t + gg) % 2 == 0:
                                nc.scalar.copy(dst, srcs[:, 3 * gg : 3 * gg + 3, :])
                            else:
                                nc.vector.tensor_copy(dst, srcs[:, 3 * gg : 3 * gg + 3, :])

                a_rows = [None] * 4
                st6s = [None] * 4
                tdone = []
                for n in range(2):
                    ns = slice(n * 384, (n + 1) * 384)
                    w2ts = []
                    for jc in range(24):
                        wt = w2_p.tile([128, 384], BF16, tag="w2", name=f"w2_{n}_{jc}")
                        nc.sync.dma_start(out=wt[:], in_=d_w2[i, jc * 128 : (jc + 1) * 128, ns])
                        w2ts.append(wt)
                    for t in (3, 0, 1, 2):
                        ps = mm_ps.tile([128, 512], F32, tag="mm", name=f"f2p{t}{n}")
                        for jc in range(24):
                            nc.tensor.matmul(
                                ps[:, :384],
                                gT[jc][:, t * 128 : (t + 1) * 128],
                                w2ts[jc][:],
                                start=(jc == 0),
                                stop=(jc == 23),
                            )
                        nc.vector.tensor_tensor(h[t][:, ns], h[t][:, ns], ps[:, :384], op=OP.add)
                        if n == 0:
                            st6s[t] = emit_stats0(t, f"a{i + 1}_")
                        elif not last:
                            a_rows[t] = emit_ln_rest(t, f"a{i + 1}_", st6s[t])
                        else:
                            # bf16 row copy + lag-1 feature transposes
                            hbt = arow_p.tile([128, C], BF16, tag="ar", name=f"hb{t}")
                            nc.scalar.copy(hbt[:], h[t][:])
                            tdone.append((t, hbt))
                            if len(tdone) >= 2:
                                emit_ftr(*tdone[-2])
                if last:
                    emit_ftr(*tdone[-1])

            # ---- final: feature-LN stats + head (hT built inline above) ----
            ps_s = mm_ps.tile([128, 512], F32, tag="mm", name="ps_s")
            ps_q = mm_ps.tile([128, 512], F32, tag="mm", name="ps_q")
            for cc in range(6):
                s = aT_p.tile([128, NTOK], BF16, tag="at", name=f"sq{cc}")
                nc.scalar.activation(s[:], hTr(cc), AF.Square)
                nc.tensor.matmul(
                    ps_s[0:1, :], onesB[:], hTr(cc), start=(cc == 0), stop=(cc == 5)
                )
                nc.tensor.matmul(
                    ps_q[0:1, :], onesB[:], s[:], start=(cc == 0), stop=(cc == 5)
                )
            sum_s = sm512_p.tile([1, SP], F32, tag="rbs", name="sum_s", bufs=1)
            nc.vector.tensor_reduce(
                sum_s[:], ps_s[0:1, :].rearrange("p (g s) -> p s g", g=TB),
                axis=AX.X, op=OP.add,
            )
            sum_q = sm512_p.tile([1, SP], F32, tag="rbs", name="sum_q", bufs=1)
            nc.vector.tensor_reduce(
                sum_q[:], ps_q[0:1, :].rearrange("p (g s) -> p s g", g=TB),
                axis=AX.X, op=OP.add,
            )
            mean = sm512_p.tile([1, SP], F32, tag="mn", name="mean")
            nc.vector.tensor_scalar_mul(mean[:], sum_s[:], 1.0 / FD)
            msq = sm512_p.tile([1, SP], F32, tag="mn", name="msq")
            nc.vector.tensor_scalar_mul(msq[:], sum_q[:], 1.0 / FD)
            mm2 = sm512_p.tile([1, SP], F32, tag="rcp", name="mm2")
            nc.vector.tensor_tensor(mm2[:], mean[:], mean[:], op=OP.mult)
            var = sm512_p.tile([1, SP], F32, tag="rcp", name="var")
            nc.vector.tensor_tensor(var[:], msq[:], mm2[:], op=OP.subtract)
            stdf = sm512_p.tile([1, SP], F32, tag="rcp", name="stdf")
            nc.scalar.activation(stdf[:], var[:], AF.Sqrt, bias=eps[0:1, :])
            rstd = sm512_p.tile([1, SP], F32, tag="rcp", name="rstdf")
            nc.vector.reciprocal_approx_fast(out=rstd[:], in_=stdf[:])
            rstdB = sm_p.tile([128, SP], F32, tag="rstdB", name="rstdB", bufs=1)
            nc.gpsimd.partition_broadcast(rstdB[:, 0:SP], rstd[:])
            cm = sm512_p.tile([1, SP], F32, tag="rcp", name="cm")
            nc.vector.tensor_tensor(cm[:], mean[:], rstd[:], op=OP.mult)
            c0 = sm512_p.tile([1, 1], F32, tag="c0", name="c0")
            nc.vector.tensor_reduce(c0[:], cm[:], axis=AX.X, op=OP.add)
            c0B = sm_p.tile([128, 1], F32, tag="c0b", name="c0B")
            nc.gpsimd.partition_broadcast(c0B[:], c0[:])

            ps_l = mm_ps.tile([128, 512], F32, tag="mm", name="ps_l")
            for idx in range(48):
                cc, tb = idx // TB, idx % TB
                g, c = idx // 12, idx % 12
                nc.tensor.matmul(
                    ps_l[0:NCLS, 0:SP],
                    hw[g][:, c * NCLS : (c + 1) * NCLS],
                    hTr(cc)[:, tb * SP : (tb + 1) * SP],
                    start=(idx == 0),
                    stop=(idx == 47),
                )
            gs = sm_p.tile([128, SP], F32, tag="gs", name="gs")
            nc.vector.tensor_tensor(gs[0:NCLS, :], ps_l[0:NCLS, 0:SP], rstdB[0:NCLS, :], op=OP.mult)
            red = sm_p.tile([128, 1], F32, tag="red", name="red")
            nc.vector.tensor_reduce(red[0:NCLS, :], gs[0:NCLS, :], axis=AX.X, op=OP.add)
            t1 = sm_p.tile([128, 1], F32, tag="t1", name="t1")
            nc.vector.tensor_scalar(
                t1[0:NCLS, :],
                cb[0:NCLS, CB_W1 : CB_W1 + 1],
                c0B[0:NCLS, :],
                None,
                op0=OP.mult,
            )
            t2 = sm_p.tile([128, 1], F32, tag="t2", name="t2")
            nc.vector.tensor_tensor(t2[0:NCLS, :], red[0:NCLS, :], t1[0:NCLS, :], op=OP.subtract)
            logits = sm_p.tile([128, 1], F32, tag="lg", name="logits")
            nc.vector.tensor_scalar(
                logits[0:NCLS, :],
                t2[0:NCLS, :],
                1.0 / SP,
                cb[0:NCLS, CB_HB : CB_HB + 1],
                op0=OP.mult,
                op1=OP.add,
            )
            nc.sync.dma_start(out=d_out[:], in_=logits[0:NCLS, :])

    nc.compile()
    return nc


def _prep_inputs(inputs):
    f = np.float32
    x = np.asarray(inputs["x"], f)
    B = x.shape[0]
    xpt = np.empty((B, PVEC, NTOK), bf16)
    for b in range(B):
        xp = x[b, 0].reshape(8, 8, 8, 8, 8, 8).transpose(0, 2, 4, 1, 3, 5).reshape(NTOK, PVEC)
        xpt[b] = np.ascontiguousarray(xp.T).astype(bf16)

    qw, kw, vw, pw = (np.asarray(inputs[k], f) for k in ("qw", "kw", "vw", "pw"))
    f1w, f2w = np.asarray(inputs["f1w"], f), np.asarray(inputs["f2w"], f)
    l1w, l1b = np.asarray(inputs["ln1_w"], f), np.asarray(inputs["ln1_b"], f)
    l2w, l2b = np.asarray(inputs["ln2_w"], f), np.asarray(inputs["ln2_b"], f)

    wq = np.ascontiguousarray((qw * l1w[:, None, :]).transpose(0, 2, 1)).astype(bf16)
    wk = np.ascontiguousarray((kw * l1w[:, None, :]).transpose(0, 2, 1)).astype(bf16)
    wv = np.ascontiguousarray(
        (vw * l1w[:, None, :] * SV).transpose(0, 2, 1)
    ).astype(bf16)
    wp = np.ascontiguousarray(pw.transpose(0, 2, 1)).astype(bf16)
    w1 = np.ascontiguousarray((f1w * l2w[:, None, :]).transpose(0, 2, 1)).astype(bf16)
    w2 = np.ascontiguousarray(f2w.transpose(0, 2, 1)).astype(bf16)

    qb = np.asarray(inputs["qb"], f) + np.einsum("ioc,ic->io", qw, l1b)
    kb = np.asarray(inputs["kb"], f) + np.einsum("ioc,ic->io", kw, l1b)
    vb = (np.asarray(inputs["vb"], f) + np.einsum("ioc,ic->io", vw, l1b)) * SV
    f1b = np.asarray(inputs["f1b"], f) + np.einsum("ijc,ic->ij", f1w, l2b)

    head_w = np.asarray(inputs["head_w"], f)
    fcn_w, fcn_b = np.asarray(inputs["fcn_w"], f), np.asarray(inputs["fcn_b"], f)
    head_b = np.asarray(inputs["head_b"], f) + head_w @ fcn_b
    hwt = np.ascontiguousarray(head_w.T * fcn_w[:, None])
    hwt_b = hwt.astype(bf16)
    # pack the 48 [128, 100] contraction chunks 24-per-tile in (cc, tb)
    # consumption order so the device loads 2 big tiles
    hwt_pk = np.zeros((4, 128, 12 * NCLS), bf16)
    for idx in range(48):
        cc, tb = idx // TB, idx % TB
        row0 = tb * C + cc * 128
        g, c = idx // 12, idx % 12
        hwt_pk[g, :, c * NCLS : (c + 1) * NCLS] = hwt_b[row0 : row0 + 128, :]

    cbp = np.zeros((128, CB_COLS), f)
    for i in range(DEPTH):
        c0 = i * CB_PER_LAYER
        cbp[:, c0 : c0 + 6] = qb[i].reshape(6, 128).T
        cbp[:, c0 + 6 : c0 + 12] = kb[i].reshape(6, 128).T
        cbp[:, c0 + 12 : c0 + 36] = f1b[i].reshape(24, 128).T
    cbp[:NCLS, CB_W1] = hwt_b.astype(f).sum(axis=0)
    cbp[:NCLS, CB_HB] = head_b

    rb = np.stack(
        [
            np.stack(
                [vb[i], np.asarray(inputs["pb"], f)[i], np.asarray(inputs["f2b"], f)[i]]
            )
            for i in range(DEPTH)
        ]
    ).astype(bf16)

    pos2 = (
        np.asarray(inputs["pos_embed"], f)[0] + np.asarray(inputs["patch_b"], f)[None, :]
    ).astype(f)
    pwt = np.ascontiguousarray(np.asarray(inputs["patch_w"], f).T).astype(bf16)

    shared = {
        "pos2": pos2,
        "pwt": pwt,
        "wq": wq,
        "wk": wk,
        "wv": wv,
        "wp": wp,
        "w1": w1,
        "w2": w2,
        "cb": cbp,
        "rb": rb,
        "hwt": hwt_pk,
    }
    return xpt, shared


_NC = None


def _get_nc():
    global _NC
    if _NC is None:
        _NC = _build()
    return _NC


def kernel(**inputs):
    nc = _get_nc()
    xpt, shared = _prep_inputs(inputs)
    B = xpt.shape[0]
    in_maps = [dict(shared, xpt=xpt[b]) for b in range(B)]
    res = run_bass_kernel_spmd(nc, in_maps, list(range(B)))
    return np.stack([res.results[b]["out"] for b in range(B)]).astype(np.float32)


# revision 13
# speedup vs baseline: 1.0063x; 1.0063x over previous
"""HSIViT forward on 8 Trainium2 NeuronCores.

Sharding: pure data parallel — batch B=8, one batch item per core, no
collectives. Each core runs the full 12-layer ViT on its (512, 768)
token activations and emits its (100,) logits row.

Host-side prep (numpy, not counted in HW exec time):
  - patch cubes extracted + transposed per batch item (xpT [512, 512])
  - all weights transposed to [c_in, c_out] for the PE's lhsT layout
  - LN1/LN2 scale+bias folded into q/k/v and fc1 weights+biases
  - v weights+bias pre-scaled by SV so the fp8 eviction needs no extra op
  - final feature-LN scale/bias folded into the classifier head
  - weights cast to bf16; patch embed + head ride bf16 too

Schedule (vs the previous revision):
  - qk projection fused into the attention pipeline per output-column
    tile mc: scores for head pair mc follow qkproj(mc+1), so the ACT
    exp stream starts early and stays hidden behind PE work.
  - score matmuls row-packed: both heads of a pair run concurrently in
    disjoint PE row groups (K=64 each), into the two banks of a
    [128,1024] psum tile; one ACT exp covers both heads' j-chunk.
  - exp output is fp8(e4m3); AV runs fp8 DoubleRow (K=256/pass) with a
    ones-column in V producing the softmax denominator; reciprocal
    batched per head pair; normalization fused into the oT eviction.
  - fc2 (and patch) iterate t in (3,0,1,2) so the last token tile's
    LN chain overlaps the other tiles' matmuls; next layer's aT
    transposes then run stall-free.
  - final head weights prefetched in 2 big DMAs; the feature transpose
    runs inline with fc2 of layer 11; a dummy sqrt warms the ACT table.
"""

import os
import sys

import numpy as np

for _p in ("/opt/trn_rl_repo", "/root/.axon_site/_ro/trn_rl_repo"):
    if _p not in sys.path and os.path.isdir(_p):
        sys.path.insert(0, _p)

import ml_dtypes  # noqa: E402

import concourse.bass as bass  # noqa: E402,F401
import concourse.mybir as mybir  # noqa: E402
import concourse.tile as tile  # noqa: E402
from concourse import bacc  # noqa: E402
from concourse.bass_utils import run_bass_kernel_spmd  # noqa: E402
from concourse.masks import make_identity  # noqa: E402

F32 = mybir.dt.float32
BF16 = mybir.dt.bfloat16
FP8 = mybir.dt.float8e4
AF = mybir.ActivationFunctionType
OP = mybir.AluOpType
AX = mybir.AxisListType
DR = mybir.MatmulPerfMode.DoubleRow

DEPTH, C, NH, HD = 12, 768, 12, 64
NTOK, PVEC = 512, 512
FF = 3072
NCLS = 100
TB, SP = 8, 64
FD = TB * C
SCALE = HD**-0.5
EPS = 1e-5
SV = 32.0  # fp8 scale on the v path (weights+bias pre-scaled host-side)
VSL = NH * (HD + 1) + 4  # 784: per-key-pair-slot v row, padded so 784%16==0

CB_PER_LAYER = 36  # qb 6 + kb 6 + f1b 24 columns
CB_W1 = DEPTH * CB_PER_LAYER
CB_HB = CB_W1 + 1
CB_COLS = CB_HB + 1

bf16 = ml_dtypes.bfloat16


def _build():
    nc = bacc.Bacc(None, target_bir_lowering=False, debug=False)

    d_xpt = nc.dram_tensor("xpt", [PVEC, NTOK], BF16, kind="ExternalInput")
    d_pos2 = nc.dram_tensor("pos2", [NTOK, C], F32, kind="ExternalInput")
    d_pwt = nc.dram_tensor("pwt", [PVEC, C], BF16, kind="ExternalInput")
    d_wq = nc.dram_tensor("wq", [DEPTH, C, C], BF16, kind="ExternalInput")
    d_wk = nc.dram_tensor("wk", [DEPTH, C, C], BF16, kind="ExternalInput")
    d_wv = nc.dram_tensor("wv", [DEPTH, C, C], BF16, kind="ExternalInput")
    d_wp = nc.dram_tensor("wp", [DEPTH, C, C], BF16, kind="ExternalInput")
    d_w1 = nc.dram_tensor("w1", [DEPTH, C, FF], BF16, kind="ExternalInput")
    d_w2 = nc.dram_tensor("w2", [DEPTH, FF, C], BF16, kind="ExternalInput")
    d_cb = nc.dram_tensor("cb", [128, CB_COLS], F32, kind="ExternalInput")
    d_rb = nc.dram_tensor("rb", [DEPTH, 3, C], BF16, kind="ExternalInput")
    d_hwt = nc.dram_tensor("hwt", [4, 128, 12 * NCLS], BF16, kind="ExternalInput")
    d_out = nc.dram_tensor("out", [NCLS], F32, kind="ExternalOutput")

    from contextlib import ExitStack

    with tile.TileContext(nc) as tc:
        with ExitStack() as ctx:
            ep = ctx.enter_context
            const = ep(tc.tile_pool(name="const", bufs=1))
            hpool = ep(tc.tile_pool(name="hpool", bufs=4))
            arow_p = ep(tc.tile_pool(name="arow", bufs=4))
            aT_p = ep(tc.tile_pool(name="atp", bufs=1))
            a2T_p = ep(tc.tile_pool(name="a2tp", bufs=6))
            qT_p = ep(tc.tile_pool(name="qtp", bufs=6))
            kT_p = ep(tc.tile_pool(name="ktp", bufs=6))
            vx_p = ep(tc.tile_pool(name="vxp", bufs=2))
            ex_p = ep(tc.tile_pool(name="exp", bufs=3))
            oT_p = ep(tc.tile_pool(name="otp", bufs=6))
            gT_p = ep(tc.tile_pool(name="gtp", bufs=24))
            qkw_p = ep(tc.tile_pool(name="qkw", bufs=12))
            vpw_p = ep(tc.tile_pool(name="vpw", bufs=9))
            w1_p = ep(tc.tile_pool(name="w1p", bufs=12))
            w2_p = ep(tc.tile_pool(name="w2p", bufs=31))
            hw_p = ep(tc.tile_pool(name="hwp", bufs=2))
            bc_p = ep(tc.tile_pool(name="bcp", bufs=2))
            rcp_p = ep(tc.tile_pool(name="rcpp", bufs=2))
            den_p = ep(tc.tile_pool(name="denp", bufs=1))
            sm_p = ep(tc.tile_pool(name="smp", bufs=8))
            sm512_p = ep(tc.tile_pool(name="sm512", bufs=2))
            mm_ps = ep(tc.tile_pool(name="mmps", bufs=3, space="PSUM"))
            sc_ps = ep(tc.tile_pool(name="scps", bufs=2, space="PSUM"))
            tp_ps = ep(tc.tile_pool(name="tpps", bufs=1, space="PSUM"))

            ident = const.tile([128, 128], F32, tag="ident", name="ident")
            make_identity(nc, ident)
            identB = const.tile([128, 128], BF16, tag="identB", name="identB")
            nc.scalar.copy(identB[:], ident[:])
            ones0 = const.tile([128, 1], F32, tag="ones0", name="ones0")
            nc.vector.memset(ones0[:], 1.0)
            onesB = const.tile([128, 1], BF16, tag="onesB", name="onesB")
            nc.scalar.copy(onesB[:], ones0[:])
            eps = const.tile([128, 1], F32, tag="eps", name="eps")
            nc.vector.memset(eps[:], EPS)
            cb = const.tile([128, CB_COLS], F32, tag="cb", name="cb")
            nc.sync.dma_start(out=cb[:], in_=d_cb[:])

            h = []
            for t in range(4):
                ht = hpool.tile([128, C], F32, tag="h", name=f"h{t}")
                h.append(ht)

            def emit_stats0(t, tag):
                st6 = sm_p.tile([128, 12], F32, tag="st6", name=f"st6_{tag}{t}")
                nc.vector.bn_stats(st6[:, 0:6], h[t][:, 0:384])
                return st6

            def emit_ln_rest(t, tag, st6):
                nc.vector.bn_stats(st6[:, 6:12], h[t][:, 384:768])
                mv = sm_p.tile([128, 2], F32, tag="mv", name=f"mv{tag}{t}")
                nc.vector.bn_aggr(mv[:], st6.rearrange("p (g s) -> p g s", g=2))
                std = sm_p.tile([128, 1], F32, tag="std", name=f"std{tag}{t}")
                nc.scalar.activation(std[:], mv[:, 1:2], AF.Sqrt, bias=eps[:])
                rstd = sm_p.tile([128, 1], F32, tag="rstd", name=f"rstd{tag}{t}")
                nc.vector.reciprocal_approx_fast(out=rstd[:], in_=std[:])
                at = arow_p.tile([128, C], BF16, tag="ar", name=f"ar{tag}{t}")
                nc.vector.tensor_scalar(
                    at[:], h[t], mv[:, 0:1], rstd[:], op0=OP.subtract, op1=OP.mult
                )
                return at

            def transpose_pass(rows, t_list, col0, outs, tag2):
                """Transpose token tiles t_list into cols [col0:] of the 6
                col tiles; evictions alternate ACT/DVE on tp-tile halves."""
                w = 128 * len(t_list)
                tp = tp_ps.tile([128, 1024], BF16, tag="tp", name=f"tp{tag2}")
                for cc in range(6):
                    sl = tp[:, (cc % 2) * 512 : (cc % 2) * 512 + 512]
                    for ti, t in enumerate(t_list):
                        nc.tensor.transpose(
                            sl[:, ti * 128 : (ti + 1) * 128],
                            rows[t][:, cc * 128 : (cc + 1) * 128],
                            identB[:],
                        )
                    dst = outs[cc][:, col0 : col0 + w]
                    if cc % 2 == 0:
                        nc.scalar.copy(dst, sl[:, 0:w])
                    else:
                        nc.vector.tensor_copy(dst, sl[:, 0:w])

            def transpose_big(rows, order, big, tag2):
                """Per token tile: 6 PE transposes + 2 strided evictions into
                the [128, 6*512] column-major tile; tp halves double-buffer."""
                bigr = big.rearrange("p (c w) -> p c w", c=6)
                tp = tp_ps.tile([128, 1024], BF16, tag="tp", name=f"tp{tag2}")
                k = 0
                for t in order:
                    for c0, ncc in ((0, 4), (4, 2)):
                        sl = tp[:, (k % 2) * 512 : (k % 2) * 512 + 128 * ncc]
                        for ci in range(ncc):
                            nc.tensor.transpose(
                                sl[:, ci * 128 : (ci + 1) * 128],
                                rows[t][:, (c0 + ci) * 128 : (c0 + ci + 1) * 128],
                                identB[:],
                            )
                        dst = bigr[:, c0 : c0 + ncc, t * 128 : (t + 1) * 128]
                        srcv = sl.rearrange("p (c w) -> p c w", c=ncc)
                        if k % 2 == 0:
                            nc.scalar.copy(dst, srcv)
                        else:
                            nc.vector.tensor_copy(dst, srcv)
                        k += 1

            def bcast_row(i, j, tag):
                src = sm512_p.tile([1, C], BF16, tag="rbs", name=f"rbs{i}_{j}", bufs=1)
                nc.sync.dma_start(out=src[:], in_=d_rb[i, j])
                bt = bc_p.tile([128, C], BF16, tag="bc", name=f"{tag}{i}")
                nc.gpsimd.partition_broadcast(bt[:], src[:])
                return bt

            # ---- patch embed: h = pos(+patch_b) + xp @ patch_w.T ----
            xpt = []
            pwt = []
            for kc in range(4):
                xt = qT_p.tile([128, NTOK], BF16, tag="qt", name=f"xpt{kc}")
                nc.sync.dma_start(out=xt[:], in_=d_xpt[kc * 128 : (kc + 1) * 128, :])
                xpt.append(xt)
            for kc in range(4):
                wt = vpw_p.tile([128, C], BF16, tag="vpw", name=f"pwt{kc}")
                nc.sync.dma_start(out=wt[:], in_=d_pwt[kc * 128 : (kc + 1) * 128, :])
                pwt.append(wt)
            a_rows = [None] * 4
            for t in (3, 0, 1, 2):
                nc.sync.dma_start(out=h[t][:], in_=d_pos2[t * 128 : (t + 1) * 128, :])
                st6 = None
                for n in range(2):
                    ns = slice(n * 384, (n + 1) * 384)
                    ps = mm_ps.tile([128, 512], F32, tag="mm", name=f"pep{t}{n}")
                    for kc in range(4):
                        nc.tensor.matmul(
                            ps[:, :384],
                            xpt[kc][:, t * 128 : (t + 1) * 128],
                            pwt[kc][:, ns],
                            start=(kc == 0),
                            stop=(kc == 3),
                        )
                    nc.vector.tensor_tensor(h[t][:, ns], h[t][:, ns], ps[:, :384], op=OP.add)
                    if n == 0:
                        st6 = emit_stats0(t, "a0_")
                a_rows[t] = emit_ln_rest(t, "a0_", st6)

            for i in range(DEPTH):
                cb0 = i * CB_PER_LAYER
                # ---- transpose all 4 LN1 token tiles (last-LN'd tile last) ----
                aT = aT_p.tile([128, 6 * NTOK], BF16, tag="at", name=f"at{i}", bufs=1)
                transpose_big(a_rows, (3, 0, 1, 2), aT, f"at{i}_")

                # ---- weights for this layer ----
                qk_w = []
                for (dw, tg) in ((d_wq, "qw"), (d_wk, "kw")):
                    wts = []
                    for kc in range(6):
                        wt = qkw_p.tile([128, C], BF16, tag="qkw", name=f"{tg}{kc}")
                        nc.sync.dma_start(out=wt[:], in_=dw[i, kc * 128 : (kc + 1) * 128, :])
                        wts.append(wt)
                    qk_w.append(wts)
                vwts = []
                for kc in range(6):
                    wt = vpw_p.tile([128, C], BF16, tag="vpw", name=f"vw{kc}")
                    nc.sync.dma_start(out=wt[:], in_=d_wv[i, kc * 128 : (kc + 1) * 128, :])
                    vwts.append(wt)
                pwts = []
                for kc in range(6):
                    wt = vpw_p.tile([128, C], BF16, tag="vpw", name=f"pw{kc}")
                    nc.sync.dma_start(out=wt[:], in_=d_wp[i, kc * 128 : (kc + 1) * 128, :])
                    pwts.append(wt)

                vbB = bcast_row(i, 0, "vb")
                pbB = bcast_row(i, 1, "pb")

                # v extended tiles: [key, slot(2), 12*(64+1)+pad] fp8, slot =
                # key-chunk within the DoubleRow pair; ones col at 64 of each
                # 65-wide head block gives the softmax denominator.
                vx = []
                for g in range(2):
                    vt = vx_p.tile([128, 2 * VSL], FP8, tag="vx", name=f"vx{g}")
                    vx.append(vt)
                    for s in range(2):
                        ones_sl = vt[:, s * VSL : s * VSL + NH * 65].rearrange(
                            "p (h d) -> p h d", h=NH
                        )[:, :, HD : HD + 1]
                        nc.vector.memset(ones_sl, 1.0)

                qT = [None] * 6
                kT = [None] * 6
                ex_all = [None] * 6
                po_all = [None] * NH
                oT = []
                for cc in range(6):
                    ot = oT_p.tile([128, NTOK], BF16, tag="ot", name=f"ot{cc}")
                    oT.append(ot)

                def emit_qkproj_one(mc, which):
                    wts, outs, base = (
                        (qk_w[0], qT, cb0) if which == 0 else (qk_w[1], kT, cb0 + 6)
                    )
                    ps = mm_ps.tile([128, 512], F32, tag="mm", name=f"qkp{mc}_{which}")
                    for k in range(6):
                        kc = (k + mc) % 6
                        nc.tensor.matmul(
                            ps[:],
                            wts[kc][:, mc * 128 : (mc + 1) * 128],
                            aT[:, kc * 512 : (kc + 1) * 512],
                            start=(k == 0),
                            stop=(k == 5),
                        )
                    out = (qT_p if which == 0 else kT_p).tile(
                        [128, NTOK], BF16, tag="qt" if which == 0 else "kt",
                        name=f"{'qk'[which]}T{mc}",
                    )
                    nc.vector.tensor_scalar_add(
                        out[:], ps[:], cb[:, base + mc : base + mc + 1]
                    )
                    outs[mc] = out

                def emit_pair_scores(p, jlist, expair):
                    # heads 2p (rows 0:64) and 2p+1 (rows 64:128) of qT/kT[p],
                    # row-packed into the two banks of a [128,1024] psum tile;
                    # exp -> fp8 ex tile laid out [j(4), hE|hO (2x512)].
                    for j in jlist:
                        sc = sc_ps.tile([128, 1024], F32, tag="sc", name=f"sc{p}_{j}")
                        for hi in range(2):
                            off = hi * 64
                            nc.tensor.matmul(
                                sc[:, hi * 512 : (hi + 1) * 512],
                                kT[p][off : off + 64, j * 128 : (j + 1) * 128],
                                qT[p][off : off + 64, :],
                                start=True,
                                stop=True,
                            )
                        nc.scalar.activation(
                            expair[:, j * 1024 : (j + 1) * 1024], sc[:], AF.Exp,
                            scale=SCALE,
                        )
                    ex_all[p] = expair

                def emit_vgroup(t):
                    vxt = vx[t // 2]
                    base = (t % 2) * VSL
                    for n in range(2):
                        ps = mm_ps.tile([128, 512], F32, tag="mm", name=f"vp{t}{n}")
                        for kc in range(6):
                            nc.tensor.matmul(
                                ps[:, :384],
                                aT[:, kc * 512 + t * 128 : kc * 512 + (t + 1) * 128],
                                vwts[kc][:, n * 384 : (n + 1) * 384],
                                start=(kc == 0),
                                stop=(kc == 5),
                            )
                        dst = vxt[:, base + n * 6 * 65 : base + (n + 1) * 6 * 65].rearrange(
                            "p (h d) -> p h d", h=6
                        )[:, :, 0:HD]
                        nc.vector.tensor_tensor(
                            dst,
                            ps[:, :384].rearrange("p (g d) -> p g d", g=6),
                            vbB[:, n * 384 : (n + 1) * 384].rearrange(
                                "p (g d) -> p g d", g=6
                            ),
                            op=OP.add,
                        )

                def emit_pair_av(p):
                    exr = ex_all[p].rearrange("q (j c) -> q j c", j=4)
                    for hi in range(2):
                        hh = 2 * p + hi
                        po = mm_ps.tile([128, 512], F32, tag="mm", name=f"po{hh}")
                        for jp in range(2):
                            nc.tensor.matmul(
                                po[0 : HD + 1, :],
                                vx[jp].rearrange("q (s c) -> q s c", s=2)[
                                    :, :, hh * 65 : hh * 65 + 65
                                ],
                                exr[:, 2 * jp : 2 * jp + 2, hi * 512 : (hi + 1) * 512],
                                start=(jp == 0),
                                stop=(jp == 1),
                                perf_mode=DR,
                            )
                        den = den_p.tile([1, NTOK], F32, tag="dr", name=f"den{hh}", bufs=2)
                        nc.vector.tensor_scalar_mul(den[:], po[HD : HD + 1, :], SV)
                        rcp = den_p.tile([1, NTOK], F32, tag="dr", name=f"rcp{hh}", bufs=2)
                        nc.vector.reciprocal_approx_fast(out=rcp[:], in_=den[:])
                        rb_ = rcp_p.tile([64, NTOK], F32, tag="rb", name=f"rcpB{hh}")
                        nc.gpsimd.partition_broadcast(rb_[:], rcp[:])
                        nc.vector.tensor_tensor(
                            oT[hh // 2][(hh % 2) * 64 : (hh % 2) * 64 + 64, :],
                            po[0:HD, :],
                            rb_[:],
                            op=OP.mult,
                        )

                # pipeline: qkproj leads scores by 1 tile; vgroups early; AV
                # lags scores by 2 pairs so exps drain on ACT.
                for mc in range(6):
                    emit_qkproj_one(mc, 0)
                    if mc >= 1:
                        ex_t = ex_p.tile([128, 4096], FP8, tag="ex", name=f"ex{mc - 1}")
                        emit_pair_scores(mc - 1, (0, 1), ex_t)
                    emit_qkproj_one(mc, 1)
                    if mc >= 1:
                        emit_pair_scores(mc - 1, (2, 3), ex_t)
                    if mc == 1:
                        emit_vgroup(0)
                        emit_vgroup(1)
                    if mc == 2:
                        emit_vgroup(2)
                        emit_vgroup(3)
                        nc.vector.tensor_tensor(h[0][:], h[0][:], pbB[:], op=OP.add)
                        nc.vector.tensor_tensor(h[1][:], h[1][:], pbB[:], op=OP.add)
                    if mc == 3:
                        nc.vector.tensor_tensor(h[2][:], h[2][:], pbB[:], op=OP.add)
                        nc.vector.tensor_tensor(h[3][:], h[3][:], pbB[:], op=OP.add)
                    if mc >= 3:
                        emit_pair_av(mc - 3)
                ex_t = ex_p.tile([128, 4096], FP8, tag="ex", name="ex5")
                emit_pair_scores(5, (0, 1, 2, 3), ex_t)
                emit_pair_av(3)
                emit_pair_av(4)
                emit_pair_av(5)

                # ---- output projection + residual (pb pre-added), LN2 inline ----
                a2_rows = [None] * 4
                for t in range(4):
                    st6 = None
                    for n in range(2):
                        ns = slice(n * 384, (n + 1) * 384)
                        ps = mm_ps.tile([128, 512], F32, tag="mm", name=f"prj{t}{n}")
                        for k in range(6):
                            kc = (k + 1) % 6
                            nc.tensor.matmul(
                                ps[:, :384],
                                oT[kc][:, t * 128 : (t + 1) * 128],
                                pwts[kc][:, ns],
                                start=(k == 0),
                                stop=(k == 5),
                            )
                        nc.vector.tensor_tensor(h[t][:, ns], h[t][:, ns], ps[:, :384], op=OP.add)
                        if n == 0:
                            st6 = emit_stats0(t, f"b{i}_")
                    a2_rows[t] = emit_ln_rest(t, f"b{i}_", st6)
                a2T = [
                    a2T_p.tile([128, NTOK], BF16, tag="a2t", name=f"a2t{cc}")
                    for cc in range(6)
                ]
                transpose_pass(a2_rows, (0, 1, 2), 0, a2T, f"a2t{i}_")

                # ---- fc1 + gelu; first half's 0:384 pass hides t3's LN+transpose ----
                f2bB = bcast_row(i, 2, "fb")
                gT = [gT_p.tile([128, NTOK], BF16, tag="gt", name=f"gt{m}") for m in range(24)]
                for half in range(2):
                    wts = []
                    for kc in range(6):
                        wt = w1_p.tile([128, FF // 2], BF16, tag="w1", name=f"w1_{half}_{kc}")
                        nc.sync.dma_start(
                            out=wt[:],
                            in_=d_w1[
                                i,
                                kc * 128 : (kc + 1) * 128,
                                half * (FF // 2) : (half + 1) * (FF // 2),
                            ],
                        )
                        wts.append(wt)
                    passes = ((0, 384), (384, 512)) if half == 0 else ((0, 512),)
                    for cs, ce in passes:
                        if cs == 384 and half == 0:
                            transpose_pass(a2_rows, (3,), 384, a2T, f"a2u{i}_")
                        w = ce - cs
                        for mh in range(12):
                            m = half * 12 + mh
                            ps = mm_ps.tile([128, 512], F32, tag="mm", name=f"f1p{m}")
                            for k in range(6):
                                kc = (k + mh) % 6
                                nc.tensor.matmul(
                                    ps[:, 0:w],
                                    wts[kc][:, mh * 128 : (mh + 1) * 128],
                                    a2T[kc][:, cs:ce],
                                    start=(k == 0),
                                    stop=(k == 5),
                                )
                            nc.scalar.activation(
                                gT[m][:, cs:ce], ps[:, 0:w], AF.Gelu,
                                bias=cb[:, cb0 + 12 + m : cb0 + 13 + m],
                            )
                        if half == 0 and cs == 0:
                            for t in range(4):
                                nc.vector.tensor_tensor(h[t][:], h[t][:], f2bB[:], op=OP.add)

                # ---- fc2 + residual; t order (3,0,1,2); next LN1 (or final
                # feature transpose for the last layer) inline ----
                last = i == DEPTH - 1
                if last:
                    # prefetch head weights; warm the sqrt table while ACT idles
                    hw = []
                    for g in range(4):
                        hwt_t = hw_p.tile([128, 12 * NCLS], BF16, tag="hw", name=f"hw{g}")
                        nc.sync.dma_start(out=hwt_t[:], in_=d_hwt[g])
                        hw.append(hwt_t)
                    dum = sm_p.tile([1, 1], F32, tag="dum", name="dum")
                    nc.scalar.activation(dum[:], eps[0:1, :], AF.Sqrt)
                    hTa = w1_p.tile([128, 3 * NTOK], BF16, tag="w1", name="hTa")
                    hTb = w1_p.tile([128, 3 * NTOK], BF16, tag="w1", name="hTb")

                    def hTr(cc):
                        t_ = hTa if cc < 3 else hTb
                        return t_.rearrange("p (c w) -> p c w", c=3)[:, cc % 3, :]

                    def emit_ftr(t, hbt):
                        tp = tp_ps.tile([128, 1024], BF16, tag="tp", name=f"tpf{t}")
                        for cc in range(6):
                            nc.tensor.transpose(
                                tp[:, cc * 128 : (cc + 1) * 128],
                                hbt[:, cc * 128 : (cc + 1) * 128],
                                identB[:],
                            )
                        srcs = tp[:, 0:768].rearrange("p (c w) -> p c w", c=6)
                        for gg, dtile in ((0, hTa), (1, hTb)):
                            dst = dtile.rearrange("p (c w) -> p c w", c=3)[
                                :, :, t * 128 : (t + 1) * 128
                            ]
                            if (t + gg) % 2 == 0:
                                nc.scalar.copy(dst, srcs[:, 3 * gg : 3 * gg + 3, :])
                            else:
                                nc.vector.tensor_copy(dst, srcs[:, 3 * gg : 3 * gg + 3, :])

                a_rows = [None] * 4
                st6s = [None] * 4
                tdone = []
                for n in range(2):
                    ns = slice(n * 384, (n + 1) * 384)
                    w2ts = []
                    for jc in range(24):
                        wt = w2_p.tile([128, 384], BF16, tag="w2", name=f"w2_{n}_{jc}")
                        nc.sync.dma_start(out=wt[:], in_=d_w2[i, jc * 128 : (jc + 1) * 128, ns])
                        w2ts.append(wt)
                    for t in (3, 0, 1, 2):
                        ps = mm_ps.tile([128, 512], F32, tag="mm", name=f"f2p{t}{n}")
                        for jc in range(24):
                            nc.tensor.matmul(
                                ps[:, :384],
                                gT[jc][:, t * 128 : (t + 1) * 128],
                                w2ts[jc][:],
                                start=(jc == 0),
                                stop=(jc == 23),
                            )
                        nc.vector.tensor_tensor(h[t][:, ns], h[t][:, ns], ps[:, :384], op=OP.add)
                        if n == 0:
                            st6s[t] = emit_stats0(t, f"a{i + 1}_")
                        elif not last:
                            a_rows[t] = emit_ln_rest(t, f"a{i + 1}_", st6s[t])
                        else:
                            # bf16 row copy + lag-1 feature transposes
                            hbt = arow_p.tile([128, C], BF16, tag="ar", name=f"hb{t}")
                            nc.scalar.copy(hbt[:], h[t][:])
                            tdone.append((t, hbt))
                            if len(tdone) >= 2:
                                emit_ftr(*tdone[-2])
                if last:
                    emit_ftr(*tdone[-1])

            # ---- final: feature-LN stats + head (hT built inline above) ----
            ps_s = mm_ps.tile([128, 512], F32, tag="mm", name="ps_s")
            ps_q = mm_ps.tile([128, 512], F32, tag="mm", name="ps_q")
            for cc in range(6):
                s = aT_p.tile([128, NTOK], BF16, tag="at", name=f"sq{cc}")
                nc.scalar.activation(s[:], hTr(cc), AF.Square)
                nc.tensor.matmul(
                    ps_s[0:1, :], onesB[:], hTr(cc), start=(cc == 0), stop=(cc == 5)
                )
                nc.tensor.matmul(
                    ps_q[0:1, :], onesB[:], s[:], start=(cc == 0), stop=(cc == 5)
                )
            sum_s = sm512_p.tile([1, SP], F32, tag="rbs", name="sum_s", bufs=1)
            nc.vector.tensor_reduce(
                sum_s[:], ps_s[0:1, :].rearrange("p (g s) -> p s g", g=TB),
                axis=AX.X, op=OP.add,
            )
            sum_q = sm512_p.tile([1, SP], F32, tag="rbs", name="sum_q", bufs=1)
            nc.vector.tensor_reduce(
                sum_q[:], ps_q[0:1, :].rearrange("p (g s) -> p s g", g=TB),
                axis=AX.X, op=OP.add,
            )
            mean = sm512_p.tile([1, SP], F32, tag="mn", name="mean")
            nc.vector.tensor_scalar_mul(mean[:], sum_s[:], 1.0 / FD)
            msq = sm512_p.tile([1, SP], F32, tag="mn", name="msq")
            nc.vector.tensor_scalar_mul(msq[:], sum_q[:], 1.0 / FD)
            mm2 = sm512_p.tile([1, SP], F32, tag="rcp", name="mm2")
            nc.vector.tensor_tensor(mm2[:], mean[:], mean[:], op=OP.mult)
            var = sm512_p.tile([1, SP], F32, tag="rcp", name="var")
            nc.vector.tensor_tensor(var[:], msq[:], mm2[:], op=OP.subtract)
            stdf = sm512_p.tile([1, SP], F32, tag="rcp", name="stdf")
            nc.scalar.activation(stdf[:], var[:], AF.Sqrt, bias=eps[0:1, :])
            rstd = sm512_p.tile([1, SP], F32, tag="rcp", name="rstdf")
            nc.vector.reciprocal_approx_fast(out=rstd[:], in_=stdf[:])
            rstdB = sm_p.tile([128, SP], F32, tag="rstdB", name="rstdB", bufs=1)
            nc.gpsimd.partition_broadcast(rstdB[:, 0:SP], rstd[:])
            cm = sm512_p.tile([1, SP], F32, tag="rcp", name="cm")
            nc.vector.tensor_tensor(cm[:], mean[:], rstd[:], op=OP.mult)
            c0 = sm512_p.tile([1, 1], F32, tag="c0", name="c0")
            nc.vector.tensor_reduce(c0[:], cm[:], axis=AX.X, op=OP.add)
            c0B = sm_p.tile([128, 1], F32, tag="c0b", name="c0B")
            nc.gpsimd.partition_broadcast(c0B[:], c0[:])

            ps_l = mm_ps.tile([128, 512], F32, tag="mm", name="ps_l")
            for idx in range(48):
                cc, tb = idx // TB, idx % TB
                g, c = idx // 12, idx % 12
                nc.tensor.matmul(
                    ps_l[0:NCLS, 0:SP],
                    hw[g][:, c * NCLS : (c + 1) * NCLS],
                    hTr(cc)[:, tb * SP : (tb + 1) * SP],
                    start=(idx == 0),
                    stop=(idx == 47),
                )
            gs = sm_p.tile([128, SP], F32, tag="gs", name="gs")
            nc.vector.tensor_tensor(gs[0:NCLS, :], ps_l[0:NCLS, 0:SP], rstdB[0:NCLS, :], op=OP.mult)
            red = sm_p.tile([128, 1], F32, tag="red", name="red")
            nc.vector.tensor_reduce(red[0:NCLS, :], gs[0:NCLS, :], axis=AX.X, op=OP.add)
            t1 = sm_p.tile([128, 1], F32, tag="t1", name="t1")
            nc.vector.tensor_scalar(
                t1[0:NCLS, :],
                cb[0:NCLS, CB_W1 : CB_W1 + 1],
                c0B[0:NCLS, :],
                None,
                op0=OP.mult,
            )
            t2 = sm_p.tile([128, 1], F32, tag="t2", name="t2")
            nc.vector.tensor_tensor(t2[0:NCLS, :], red[0:NCLS, :], t1[0:NCLS, :], op=OP.subtract)
            logits = sm_p.tile([128, 1], F32, tag="lg", name="logits")
            nc.vector.tensor_scalar(
                logits[0:NCLS, :],
                t2[0:NCLS, :],
                1.0 / SP,
                cb[0:NCLS, CB_HB : CB_HB + 1],
                op0=OP.mult,
                op1=OP.add,
            )
            nc.sync.dma_start(out=d_out[:], in_=logits[0:NCLS, :])

    nc.compile()
    return nc


def _prep_inputs(inputs):
    f = np.float32
    x = np.asarray(inputs["x"], f)
    B = x.shape[0]
    xpt = np.empty((B, PVEC, NTOK), bf16)
    for b in range(B):
        xp = x[b, 0].reshape(8, 8, 8, 8, 8, 8).transpose(0, 2, 4, 1, 3, 5).reshape(NTOK, PVEC)
        xpt[b] = np.ascontiguousarray(xp.T).astype(bf16)

    qw, kw, vw, pw = (np.asarray(inputs[k], f) for k in ("qw", "kw", "vw", "pw"))
    f1w, f2w = np.asarray(inputs["f1w"], f), np.asarray(inputs["f2w"], f)
    l1w, l1b = np.asarray(inputs["ln1_w"], f), np.asarray(inputs["ln1_b"], f)
    l2w, l2b = np.asarray(inputs["ln2_w"], f), np.asarray(inputs["ln2_b"], f)

    wq = np.ascontiguousarray((qw * l1w[:, None, :]).transpose(0, 2, 1)).astype(bf16)
    wk = np.ascontiguousarray((kw * l1w[:, None, :]).transpose(0, 2, 1)).astype(bf16)
    wv = np.ascontiguousarray(
        (vw * l1w[:, None, :] * SV).transpose(0, 2, 1)
    ).astype(bf16)
    wp = np.ascontiguousarray(pw.transpose(0, 2, 1)).astype(bf16)
    w1 = np.ascontiguousarray((f1w * l2w[:, None, :]).transpose(0, 2, 1)).astype(bf16)
    w2 = np.ascontiguousarray(f2w.transpose(0, 2, 1)).astype(bf16)

    qb = np.asarray(inputs["qb"], f) + np.einsum("ioc,ic->io", qw, l1b)
    kb = np.asarray(inputs["kb"], f) + np.einsum("ioc,ic->io", kw, l1b)
    vb = (np.asarray(inputs["vb"], f) + np.einsum("ioc,ic->io", vw, l1b)) * SV
    f1b = np.asarray(inputs["f1b"], f) + np.einsum("ijc,ic->ij", f1w, l2b)

    head_w = np.asarray(inputs["head_w"], f)
    fcn_w, fcn_b = np.asarray(inputs["fcn_w"], f), np.asarray(inputs["fcn_b"], f)
    head_b = np.asarray(inputs["head_b"], f) + head_w @ fcn_b
    hwt = np.ascontiguousarray(head_w.T * fcn_w[:, None])
    hwt_b = hwt.astype(bf16)
    # pack the 48 [128, 100] contraction chunks 24-per-tile in (cc, tb)
    # consumption order so the device loads 2 big tiles
    hwt_pk = np.zeros((4, 128, 12 * NCLS), bf16)
    for idx in range(48):
        cc, tb = idx // TB, idx % TB
        row0 = tb * C + cc * 128
        g, c = idx // 12, idx % 12
        hwt_pk[g, :, c * NCLS : (c + 1) * NCLS] = hwt_b[row0 : row0 + 128, :]

    cbp = np.zeros((128, CB_COLS), f)
    for i in range(DEPTH):
        c0 = i * CB_PER_LAYER
        cbp[:, c0 : c0 + 6] = qb[i].reshape(6, 128).T
        cbp[:, c0 + 6 : c0 + 12] = kb[i].reshape(6, 128).T
        cbp[:, c0 + 12 : c0 + 36] = f1b[i].reshape(24, 128).T
    cbp[:NCLS, CB_W1] = hwt_b.astype(f).sum(axis=0)
    cbp[:NCLS, CB_HB] = head_b

    rb = np.stack(
        [
            np.stack(
                [vb[i], np.asarray(inputs["pb"], f)[i], np.asarray(inputs["f2b"], f)[i]]
            )
            for i in range(DEPTH)
        ]
    ).astype(bf16)

    pos2 = (
        np.asarray(inputs["pos_embed"], f)[0] + np.asarray(inputs["patch_b"], f)[None, :]
    ).astype(f)
    pwt = np.ascontiguousarray(np.asarray(inputs["patch_w"], f).T).astype(bf16)

    shared = {
        "pos2": pos2,
        "pwt": pwt,
        "wq": wq,
        "wk": wk,
        "wv": wv,
        "wp": wp,
        "w1": w1,
        "w2": w2,
        "cb": cbp,
        "rb": rb,
        "hwt": hwt_pk,
    }
    return xpt, shared


_NC = None


def _get_nc():
    global _NC
    if _NC is None:
        _NC = _build()
    return _NC


def kernel(**inputs):
    nc = _get_nc()
    xpt, shared = _prep_inputs(inputs)
    B = xpt.shape[0]
    in_maps = [dict(shared, xpt=xpt[b]) for b in range(B)]
    res = run_bass_kernel_spmd(nc, in_maps, list(range(B)))
    return np.stack([res.results[b]["out"] for b in range(B)]).astype(np.float32)


# revision 15
# speedup vs baseline: 1.0290x; 1.0226x over previous
"""HSIViT forward on 8 Trainium2 NeuronCores.

Sharding: pure data parallel — batch B=8, one batch item per core, no
collectives. Each core runs the full 12-layer ViT on its (512, 768)
token activations and emits its (100,) logits row.

Host-side prep (numpy, not counted in HW exec time):
  - patch cubes extracted + transposed per batch item (xpT [512, 512])
  - all weights transposed to [c_in, c_out] for the PE's lhsT layout
  - LN1/LN2 scale+bias folded into q/k/v and fc1 weights+biases
  - v weights+bias pre-scaled by SV so the fp8 eviction needs no extra op
  - final feature-LN scale/bias folded into the classifier head
  - weights cast to bf16; patch embed + head ride bf16 too

Schedule (vs the previous revision):
  - qk projection fused into the attention pipeline per output-column
    tile mc: scores for head pair mc follow qkproj(mc+1), so the ACT
    exp stream starts early and stays hidden behind PE work.
  - score matmuls row-packed: both heads of a pair run concurrently in
    disjoint PE row groups (K=64 each), into the two banks of a
    [128,1024] psum tile; one ACT exp covers both heads' j-chunk.
  - exp output is fp8(e4m3); AV runs fp8 DoubleRow (K=256/pass) with a
    ones-column in V producing the softmax denominator; reciprocal
    batched per head pair; normalization fused into the oT eviction.
  - fc2 (and patch) iterate t in (3,0,1,2) so the last token tile's
    LN chain overlaps the other tiles' matmuls; next layer's aT
    transposes then run stall-free.
  - final head weights prefetched in 2 big DMAs; the feature transpose
    runs inline with fc2 of layer 11; a dummy sqrt warms the ACT table.
"""

import os
import sys

import numpy as np

for _p in ("/opt/trn_rl_repo", "/root/.axon_site/_ro/trn_rl_repo"):
    if _p not in sys.path and os.path.isdir(_p):
        sys.path.insert(0, _p)

import ml_dtypes  # noqa: E402

import concourse.bass as bass  # noqa: E402,F401
import concourse.mybir as mybir  # noqa: E402
import concourse.tile as tile  # noqa: E402
from concourse import bacc  # noqa: E402
from concourse.bass_utils import run_bass_kernel_spmd  # noqa: E402
from concourse.masks import make_identity  # noqa: E402

F32 = mybir.dt.float32
BF16 = mybir.dt.bfloat16
FP8 = mybir.dt.float8e4
AF = mybir.ActivationFunctionType
OP = mybir.AluOpType
AX = mybir.AxisListType
DR = mybir.MatmulPerfMode.DoubleRow

DEPTH, C, NH, HD = 12, 768, 12, 64
NTOK, PVEC = 512, 512
FF = 3072
NCLS = 100
TB, SP = 8, 64
FD = TB * C
SCALE = HD**-0.5
EPS = 1e-5
SV = 32.0  # fp8 scale on the v path (weights+bias pre-scaled host-side)
VSL = NH * (HD + 1) + 4  # 784: per-key-pair-slot v row, padded so 784%16==0

CB_PER_LAYER = 36  # qb 6 + kb 6 + f1b 24 columns
CB_W1 = DEPTH * CB_PER_LAYER
CB_HB = CB_W1 + 1
CB_COLS = CB_HB + 1

bf16 = ml_dtypes.bfloat16


def _build():
    nc = bacc.Bacc(None, target_bir_lowering=False, debug=False)

    d_xpt = nc.dram_tensor("xpt", [PVEC, NTOK], BF16, kind="ExternalInput")
    d_pos2 = nc.dram_tensor("pos2", [NTOK, C], F32, kind="ExternalInput")
    d_pwt = nc.dram_tensor("pwt", [PVEC, C], BF16, kind="ExternalInput")
    d_wq = nc.dram_tensor("wq", [DEPTH, C, C], BF16, kind="ExternalInput")
    d_wk = nc.dram_tensor("wk", [DEPTH, C, C], BF16, kind="ExternalInput")
    d_wv = nc.dram_tensor("wv", [DEPTH, C, C], BF16, kind="ExternalInput")
    d_wp = nc.dram_tensor("wp", [DEPTH, C, C], BF16, kind="ExternalInput")
    d_w1 = nc.dram_tensor("w1", [DEPTH, C, FF], BF16, kind="ExternalInput")
    d_w2 = nc.dram_tensor("w2", [DEPTH, FF, C], BF16, kind="ExternalInput")
    d_cb = nc.dram_tensor("cb", [128, CB_COLS], F32, kind="ExternalInput")
    d_rb = nc.dram_tensor("rb", [DEPTH, 3, C], BF16, kind="ExternalInput")
    d_hwt = nc.dram_tensor("hwt", [4, 128, 12 * NCLS], BF16, kind="ExternalInput")
    d_out = nc.dram_tensor("out", [NCLS], F32, kind="ExternalOutput")

    from contextlib import ExitStack

    with tile.TileContext(nc) as tc:
        with ExitStack() as ctx:
            ep = ctx.enter_context
            const = ep(tc.tile_pool(name="const", bufs=1))
            hpool = ep(tc.tile_pool(name="hpool", bufs=4))
            arow_p = ep(tc.tile_pool(name="arow", bufs=4))
            aT_p = ep(tc.tile_pool(name="atp", bufs=1))
            a2T_p = ep(tc.tile_pool(name="a2tp", bufs=6))
            qT_p = ep(tc.tile_pool(name="qtp", bufs=6))
            kT_p = ep(tc.tile_pool(name="ktp", bufs=6))
            vx_p = ep(tc.tile_pool(name="vxp", bufs=2))
            ex_p = ep(tc.tile_pool(name="exp", bufs=3))
            oT_p = ep(tc.tile_pool(name="otp", bufs=6))
            gT_p = ep(tc.tile_pool(name="gtp", bufs=24))
            qkw_p = ep(tc.tile_pool(name="qkw", bufs=12))
            vpw_p = ep(tc.tile_pool(name="vpw", bufs=9))
            w1_p = ep(tc.tile_pool(name="w1p", bufs=12))
            w2_p = ep(tc.tile_pool(name="w2p", bufs=31))
            hw_p = ep(tc.tile_pool(name="hwp", bufs=2))
            bc_p = ep(tc.tile_pool(name="bcp", bufs=2))
            rcp_p = ep(tc.tile_pool(name="rcpp", bufs=2))
            den_p = ep(tc.tile_pool(name="denp", bufs=1))
            sm_p = ep(tc.tile_pool(name="smp", bufs=8))
            sm512_p = ep(tc.tile_pool(name="sm512", bufs=2))
            mm_ps = ep(tc.tile_pool(name="mmps", bufs=3, space="PSUM"))
            sc_ps = ep(tc.tile_pool(name="scps", bufs=2, space="PSUM"))
            tp_ps = ep(tc.tile_pool(name="tpps", bufs=1, space="PSUM"))

            ident = const.tile([128, 128], F32, tag="ident", name="ident")
            make_identity(nc, ident)
            identB = const.tile([128, 128], BF16, tag="identB", name="identB")
            nc.scalar.copy(identB[:], ident[:])
            ones0 = const.tile([128, 1], F32, tag="ones0", name="ones0")
            nc.vector.memset(ones0[:], 1.0)
            onesB = const.tile([128, 1], BF16, tag="onesB", name="onesB")
            nc.scalar.copy(onesB[:], ones0[:])
            eps = const.tile([128, 1], F32, tag="eps", name="eps")
            nc.vector.memset(eps[:], EPS)
            cb = const.tile([128, CB_COLS], F32, tag="cb", name="cb")
            nc.sync.dma_start(out=cb[:], in_=d_cb[:])

            h = []
            for t in range(4):
                ht = hpool.tile([128, C], F32, tag="h", name=f"h{t}")
                h.append(ht)

            def emit_stats0(t, tag):
                st6 = sm_p.tile([128, 12], F32, tag="st6", name=f"st6_{tag}{t}")
                nc.vector.bn_stats(st6[:, 0:6], h[t][:, 0:384])
                return st6

            def emit_ln_rest(t, tag, st6):
                nc.vector.bn_stats(st6[:, 6:12], h[t][:, 384:768])
                mv = sm_p.tile([128, 2], F32, tag="mv", name=f"mv{tag}{t}")
                nc.vector.bn_aggr(mv[:], st6.rearrange("p (g s) -> p g s", g=2))
                std = sm_p.tile([128, 1], F32, tag="std", name=f"std{tag}{t}")
                nc.scalar.activation(std[:], mv[:, 1:2], AF.Sqrt, bias=eps[:])
                rstd = sm_p.tile([128, 1], F32, tag="rstd", name=f"rstd{tag}{t}")
                nc.vector.reciprocal_approx_fast(out=rstd[:], in_=std[:])
                at = arow_p.tile([128, C], BF16, tag="ar", name=f"ar{tag}{t}")
                nc.vector.tensor_scalar(
                    at[:], h[t], mv[:, 0:1], rstd[:], op0=OP.subtract, op1=OP.mult
                )
                return at

            def transpose_pass(rows, t_list, col0, outs, tag2):
                """Transpose token tiles t_list into cols [col0:] of the 6
                col tiles; evictions alternate ACT/DVE on tp-tile halves."""
                w = 128 * len(t_list)
                tp = tp_ps.tile([128, 1024], BF16, tag="tp", name=f"tp{tag2}")
                for cc in range(6):
                    sl = tp[:, (cc % 2) * 512 : (cc % 2) * 512 + 512]
                    for ti, t in enumerate(t_list):
                        nc.tensor.transpose(
                            sl[:, ti * 128 : (ti + 1) * 128],
                            rows[t][:, cc * 128 : (cc + 1) * 128],
                            identB[:],
                        )
                    dst = outs[cc][:, col0 : col0 + w]
                    if cc % 2 == 0:
                        nc.scalar.copy(dst, sl[:, 0:w])
                    else:
                        nc.vector.tensor_copy(dst, sl[:, 0:w])

            def transpose_big(rows, order, big, tag2):
                """Per token tile: 6 PE transposes + 2 strided evictions into
                the [128, 6*512] column-major tile; tp halves double-buffer."""
                bigr = big.rearrange("p (c w) -> p c w", c=6)
                tp = tp_ps.tile([128, 1024], BF16, tag="tp", name=f"tp{tag2}")
                k = 0
                for t in order:
                    for c0, ncc in ((0, 4), (4, 2)):
                        sl = tp[:, (k % 2) * 512 : (k % 2) * 512 + 128 * ncc]
                        for ci in range(ncc):
                            nc.tensor.transpose(
                                sl[:, ci * 128 : (ci + 1) * 128],
                                rows[t][:, (c0 + ci) * 128 : (c0 + ci + 1) * 128],
                                identB[:],
                            )
                        dst = bigr[:, c0 : c0 + ncc, t * 128 : (t + 1) * 128]
                        srcv = sl.rearrange("p (c w) -> p c w", c=ncc)
                        if k % 2 == 0:
                            nc.scalar.copy(dst, srcv)
                        else:
                            nc.vector.tensor_copy(dst, srcv)
                        k += 1

            def bcast_row(i, j, tag):
                src = sm512_p.tile([1, C], BF16, tag="rbs", name=f"rbs{i}_{j}", bufs=1)
                nc.sync.dma_start(out=src[:], in_=d_rb[i, j])
                bt = bc_p.tile([128, C], BF16, tag="bc", name=f"{tag}{i}")
                nc.gpsimd.partition_broadcast(bt[:], src[:])
                return bt

            # persistent transpose psum tile; halves double-buffer globally
            tpb = tp_ps.tile([128, 1024], BF16, tag="tp", name="tpb")
            tp_k = [0]

            def transpose_one(row_t, t, big, tag2):
                """6 PE transposes of one token tile into the [128, 6*512]
                column-major tile; 2 strided evictions alternate ACT/DVE."""
                bigr = big.rearrange("p (c w) -> p c w", c=6)
                for c0, ncc in ((0, 4), (4, 2)):
                    k = tp_k[0]
                    tp_k[0] += 1
                    sl = tpb[:, (k % 2) * 512 : (k % 2) * 512 + 128 * ncc]
                    for ci in range(ncc):
                        nc.tensor.transpose(
                            sl[:, ci * 128 : (ci + 1) * 128],
                            row_t[:, (c0 + ci) * 128 : (c0 + ci + 1) * 128],
                            identB[:],
                        )
                    dst = bigr[:, c0 : c0 + ncc, t * 128 : (t + 1) * 128]
                    srcv = sl.rearrange("p (c w) -> p c w", c=ncc)
                    if k % 2 == 0:
                        nc.scalar.copy(dst, srcv)
                    else:
                        nc.vector.tensor_copy(dst, srcv)

            # ---- patch embed: h = pos(+patch_b) + xp @ patch_w.T ----
            # lag-1 LN + transposes into layer 0's aT; t3 left pending.
            xpt = []
            pwt = []
            for kc in range(4):
                xt = qT_p.tile([128, NTOK], BF16, tag="qt", name=f"xpt{kc}")
                nc.sync.dma_start(out=xt[:], in_=d_xpt[kc * 128 : (kc + 1) * 128, :])
                xpt.append(xt)
            for kc in range(4):
                wt = vpw_p.tile([128, C], BF16, tag="vpw", name=f"pwt{kc}")
                nc.sync.dma_start(out=wt[:], in_=d_pwt[kc * 128 : (kc + 1) * 128, :])
                pwt.append(wt)
            a_rows = [None] * 4
            aT = aT_p.tile([128, 6 * NTOK], BF16, tag="at", name="at_l0", bufs=1)
            for t in range(4):
                nc.sync.dma_start(out=h[t][:], in_=d_pos2[t * 128 : (t + 1) * 128, :])
                st6 = None
                for n in range(2):
                    ns = slice(n * 384, (n + 1) * 384)
                    ps = mm_ps.tile([128, 512], F32, tag="mm", name=f"pep{t}{n}")
                    for kc in range(4):
                        nc.tensor.matmul(
                            ps[:, :384],
                            xpt[kc][:, t * 128 : (t + 1) * 128],
                            pwt[kc][:, ns],
                            start=(kc == 0),
                            stop=(kc == 3),
                        )
                    nc.vector.tensor_tensor(h[t][:, ns], h[t][:, ns], ps[:, :384], op=OP.add)
                    if n == 0:
                        st6 = emit_stats0(t, "a0_")
                a_rows[t] = emit_ln_rest(t, "a0_", st6)
                if t >= 1:
                    transpose_one(a_rows[t - 1], t - 1, aT, "pa")
            transpose_one(a_rows[2], 2, aT, "pb")
            # a_rows[3] transpose pending; done at layer-0 boundary

            for i in range(DEPTH):
                cb0 = i * CB_PER_LAYER
                # ---- weights for this layer ----
                qk_w = []
                for (dw, tg) in ((d_wq, "qw"), (d_wk, "kw")):
                    wts = []
                    for kc in range(6):
                        wt = qkw_p.tile([128, C], BF16, tag="qkw", name=f"{tg}{kc}")
                        nc.sync.dma_start(out=wt[:], in_=dw[i, kc * 128 : (kc + 1) * 128, :])
                        wts.append(wt)
                    qk_w.append(wts)
                vwts = []
                for kc in range(6):
                    wt = vpw_p.tile([128, C], BF16, tag="vpw", name=f"vw{kc}")
                    nc.sync.dma_start(out=wt[:], in_=d_wv[i, kc * 128 : (kc + 1) * 128, :])
                    vwts.append(wt)
                pwts = []
                for kc in range(6):
                    wt = vpw_p.tile([128, C], BF16, tag="vpw", name=f"pw{kc}")
                    nc.sync.dma_start(out=wt[:], in_=d_wp[i, kc * 128 : (kc + 1) * 128, :])
                    pwts.append(wt)

                vbB = bcast_row(i, 0, "vb")
                pbB = bcast_row(i, 1, "pb")

                vx = []
                for g in range(2):
                    vt = vx_p.tile([128, 2 * VSL], FP8, tag="vx", name=f"vx{g}")
                    vx.append(vt)
                    for s in range(2):
                        ones_sl = vt[:, s * VSL : s * VSL + NH * 65].rearrange(
                            "p (h d) -> p h d", h=NH
                        )[:, :, HD : HD + 1]
                        nc.vector.memset(ones_sl, 1.0)

                qT = [None] * 6
                kT = [None] * 6
                ex_all = [None] * 6
                oT = []
                for cc in range(6):
                    ot = oT_p.tile([128, NTOK], BF16, tag="ot", name=f"ot{cc}")
                    oT.append(ot)

                def qkproj0a(which):
                    wts, base = (qk_w[0], cb0) if which == 0 else (qk_w[1], cb0 + 6)
                    out = (qT_p if which == 0 else kT_p).tile(
                        [128, NTOK], BF16, tag="qt" if which == 0 else "kt",
                        name=f"{'qk'[which]}T0",
                    )
                    ps = mm_ps.tile([128, 512], F32, tag="mm", name=f"qk0a_{which}")
                    for k in range(6):
                        nc.tensor.matmul(
                            ps[:, 0:384],
                            wts[k][:, 0:128],
                            aT[:, k * 512 : k * 512 + 384],
                            start=(k == 0),
                            stop=(k == 5),
                        )
                    nc.vector.tensor_scalar_add(
                        out[:, 0:384], ps[:, 0:384], cb[:, base : base + 1]
                    )
                    return out

                def qkproj0b(which, out):
                    wts, base = (qk_w[0], cb0) if which == 0 else (qk_w[1], cb0 + 6)
                    ps = mm_ps.tile([128, 512], F32, tag="mm", name=f"qk0b_{which}")
                    for k in range(6):
                        nc.tensor.matmul(
                            ps[:, 0:128],
                            wts[k][:, 0:128],
                            aT[:, k * 512 + 384 : k * 512 + 512],
                            start=(k == 0),
                            stop=(k == 5),
                        )
                    nc.vector.tensor_scalar_add(
                        out[:, 384:512], ps[:, 0:128], cb[:, base : base + 1]
                    )

                def emit_qkproj_one(mc, which):
                    wts, outs, base = (
                        (qk_w[0], qT, cb0) if which == 0 else (qk_w[1], kT, cb0 + 6)
                    )
                    ps = mm_ps.tile([128, 512], F32, tag="mm", name=f"qkp{mc}_{which}")
                    for k in range(6):
                        kc = (k + mc) % 6
                        nc.tensor.matmul(
                            ps[:],
                            wts[kc][:, mc * 128 : (mc + 1) * 128],
                            aT[:, kc * 512 : (kc + 1) * 512],
                            start=(k == 0),
                            stop=(k == 5),
                        )
                    out = (qT_p if which == 0 else kT_p).tile(
                        [128, NTOK], BF16, tag="qt" if which == 0 else "kt",
                        name=f"{'qk'[which]}T{mc}",
                    )
                    nc.vector.tensor_scalar_add(
                        out[:], ps[:], cb[:, base + mc : base + mc + 1]
                    )
                    outs[mc] = out

                def emit_pair_scores(p, jlist, expair):
                    for j in jlist:
                        sc = sc_ps.tile([128, 1024], F32, tag="sc", name=f"sc{p}_{j}")
                        for hi in range(2):
                            off = hi * 64
                            nc.tensor.matmul(
                                sc[:, hi * 512 : (hi + 1) * 512],
                                kT[p][off : off + 64, j * 128 : (j + 1) * 128],
                                qT[p][off : off + 64, :],
                                start=True,
                                stop=True,
                            )
                        nc.scalar.activation(
                            expair[:, j * 1024 : (j + 1) * 1024], sc[:], AF.Exp,
                            scale=SCALE,
                        )
                    ex_all[p] = expair

                def emit_vgroup(t):
                    vxt = vx[t // 2]
                    base = (t % 2) * VSL
                    for n in range(2):
                        ps = mm_ps.tile([128, 512], F32, tag="mm", name=f"vp{t}{n}")
                        for kc in range(6):
                            nc.tensor.matmul(
                                ps[:, :384],
                                aT[:, kc * 512 + t * 128 : kc * 512 + (t + 1) * 128],
                                vwts[kc][:, n * 384 : (n + 1) * 384],
                                start=(kc == 0),
                                stop=(kc == 5),
                            )
                        dst = vxt[:, base + n * 6 * 65 : base + (n + 1) * 6 * 65].rearrange(
                            "p (h d) -> p h d", h=6
                        )[:, :, 0:HD]
                        nc.vector.tensor_tensor(
                            dst,
                            ps[:, :384].rearrange("p (g d) -> p g d", g=6),
                            vbB[:, n * 384 : (n + 1) * 384].rearrange(
                                "p (g d) -> p g d", g=6
                            ),
                            op=OP.add,
                        )

                def emit_pair_av(p):
                    exr = ex_all[p].rearrange("q (j c) -> q j c", j=4)
                    for hi in range(2):
                        hh = 2 * p + hi
                        po = mm_ps.tile([128, 512], F32, tag="mm", name=f"po{hh}")
                        for jp in range(2):
                            nc.tensor.matmul(
                                po[0 : HD + 1, :],
                                vx[jp].rearrange("q (s c) -> q s c", s=2)[
                                    :, :, hh * 65 : hh * 65 + 65
                                ],
                                exr[:, 2 * jp : 2 * jp + 2, hi * 512 : (hi + 1) * 512],
                                start=(jp == 0),
                                stop=(jp == 1),
                                perf_mode=DR,
                            )
                        den = den_p.tile([1, NTOK], F32, tag="dr", name=f"den{hh}", bufs=2)
                        nc.vector.tensor_scalar_mul(den[:], po[HD : HD + 1, :], SV)
                        rcp = den_p.tile([1, NTOK], F32, tag="dr", name=f"rcp{hh}", bufs=2)
                        nc.vector.reciprocal_approx_fast(out=rcp[:], in_=den[:])
                        rb_ = rcp_p.tile([64, NTOK], F32, tag="rb", name=f"rcpB{hh}")
                        nc.gpsimd.partition_broadcast(rb_[:], rcp[:])
                        nc.vector.tensor_tensor(
                            oT[hh // 2][(hh % 2) * 64 : (hh % 2) * 64 + 64, :],
                            po[0:HD, :],
                            rb_[:],
                            op=OP.mult,
                        )

                # ---- boundary: split qk-proj for mc=0 around the pending
                # t3 transpose, then the mc pipeline ----
                out_q0 = qkproj0a(0)
                out_k0 = qkproj0a(1)
                transpose_one(a_rows[3], 3, aT, f"bd{i}")
                qkproj0b(0, out_q0)
                qkproj0b(1, out_k0)
                qT[0] = out_q0
                kT[0] = out_k0

                for mc in range(1, 6):
                    emit_qkproj_one(mc, 0)
                    ex_t = ex_p.tile([128, 4096], FP8, tag="ex", name=f"ex{mc - 1}")
                    emit_pair_scores(mc - 1, (0, 1), ex_t)
                    emit_qkproj_one(mc, 1)
                    emit_pair_scores(mc - 1, (2, 3), ex_t)
                    if mc == 1:
                        emit_vgroup(0)
                        emit_vgroup(1)
                    if mc == 2:
                        emit_vgroup(2)
                        emit_vgroup(3)
                        nc.vector.tensor_tensor(h[0][:], h[0][:], pbB[:], op=OP.add)
                        nc.vector.tensor_tensor(h[1][:], h[1][:], pbB[:], op=OP.add)
                    if mc == 3:
                        nc.vector.tensor_tensor(h[2][:], h[2][:], pbB[:], op=OP.add)
                        nc.vector.tensor_tensor(h[3][:], h[3][:], pbB[:], op=OP.add)
                        emit_pair_av(0)
                    if mc == 4:
                        emit_pair_av(1)
                        emit_pair_av(2)
                    if mc == 5:
                        emit_pair_av(3)
                ex_t = ex_p.tile([128, 4096], FP8, tag="ex", name="ex5")
                emit_pair_scores(5, (0, 1), ex_t)
                emit_pair_scores(5, (2, 3), ex_t)
                emit_pair_av(4)
                emit_pair_av(5)

                # ---- output projection + residual (pb pre-added), lag-1 LN2
                # transposes into a2T; t3 pending into fc1 ----
                a2T = a2T_p.tile([128, 6 * NTOK], BF16, tag="a2t", name=f"a2t{i}", bufs=1)
                a2_rows = [None] * 4
                for t in range(4):
                    st6 = None
                    for n in range(2):
                        ns = slice(n * 384, (n + 1) * 384)
                        ps = mm_ps.tile([128, 512], F32, tag="mm", name=f"prj{t}{n}")
                        for k in range(6):
                            nc.tensor.matmul(
                                ps[:, :384],
                                oT[k][:, t * 128 : (t + 1) * 128],
                                pwts[k][:, ns],
                                start=(k == 0),
                                stop=(k == 5),
                            )
                        nc.vector.tensor_tensor(h[t][:, ns], h[t][:, ns], ps[:, :384], op=OP.add)
                        if n == 0:
                            st6 = emit_stats0(t, f"b{i}_")
                    a2_rows[t] = emit_ln_rest(t, f"b{i}_", st6)
                    if t >= 1:
                        transpose_one(a2_rows[t - 1], t - 1, a2T, f"p{i}_")
                transpose_one(a2_rows[2], 2, a2T, f"p{i}b_")

                # ---- fc1 + gelu; the 0:384 pass hides t3's LN + transpose ----
                f2bB = bcast_row(i, 2, "fb")
                gT = [gT_p.tile([128, NTOK], BF16, tag="gt", name=f"gt{m}") for m in range(24)]
                for half in range(2):
                    wts = []
                    for kc in range(6):
                        wt = w1_p.tile([128, FF // 2], BF16, tag="w1", name=f"w1_{half}_{kc}")
                        nc.sync.dma_start(
                            out=wt[:],
                            in_=d_w1[
                                i,
                                kc * 128 : (kc + 1) * 128,
                                half * (FF // 2) : (half + 1) * (FF // 2),
                            ],
                        )
                        wts.append(wt)
                    passes = ((0, 384), (384, 512)) if half == 0 else ((0, 512),)
                    for cs, ce in passes:
                        w = ce - cs
                        for mh in range(12):
                            m = half * 12 + mh
                            ps = mm_ps.tile([128, 512], F32, tag="mm", name=f"f1p{m}")
                            for k in range(6):
                                kc = (k + mh) % 6
                                nc.tensor.matmul(
                                    ps[:, 0:w],
                                    wts[kc][:, mh * 128 : (mh + 1) * 128],
                                    a2T[:, kc * 512 + cs : kc * 512 + ce],
                                    start=(k == 0),
                                    stop=(k == 5),
                                )
                            nc.scalar.activation(
                                gT[m][:, cs:ce], ps[:, 0:w], AF.Gelu,
                                bias=cb[:, cb0 + 12 + m : cb0 + 13 + m],
                            )
                            if half == 0 and cs == 0 and mh == 2:
                                transpose_one(a2_rows[3], 3, a2T, f"p{i}c_")
                        if half == 0 and cs == 0:
                            for t in range(4):
                                nc.vector.tensor_tensor(h[t][:], h[t][:], f2bB[:], op=OP.add)

                # ---- fc2 + residual; lag-1 next-layer LN1 + aT transposes
                # (or the final feature transpose on the last layer) ----
                last = i == DEPTH - 1
                if last:
                    hw = []
                    for g in range(4):
                        hwt_t = hw_p.tile([128, 12 * NCLS], BF16, tag="hw", name=f"hw{g}")
                        nc.sync.dma_start(out=hwt_t[:], in_=d_hwt[g])
                        hw.append(hwt_t)
                    dum = sm_p.tile([1, 1], F32, tag="dum", name="dum")
                    nc.scalar.activation(dum[:], eps[0:1, :], AF.Sqrt)
                    hTa = w1_p.tile([128, 3 * NTOK], BF16, tag="w1", name="hTa")
                    hTb = w1_p.tile([128, 3 * NTOK], BF16, tag="w1", name="hTb")

                    def hTr(cc):
                        t_ = hTa if cc < 3 else hTb
                        return t_.rearrange("p (c w) -> p c w", c=3)[:, cc % 3, :]

                    def emit_ftr(t, hbt):
                        for c0, dtile in ((0, hTa), (3, hTb)):
                            k = tp_k[0]
                            tp_k[0] += 1
                            sl = tpb[:, (k % 2) * 512 : (k % 2) * 512 + 384]
                            for ci in range(3):
                                nc.tensor.transpose(
                                    sl[:, ci * 128 : (ci + 1) * 128],
                                    hbt[:, (c0 + ci) * 128 : (c0 + ci + 1) * 128],
                                    identB[:],
                                )
                            dst = dtile.rearrange("p (c w) -> p c w", c=3)[
                                :, :, t * 128 : (t + 1) * 128
                            ]
                            srcv = sl.rearrange("p (c w) -> p c w", c=3)
                            if k % 2 == 0:
                                nc.scalar.copy(dst, srcv)
                            else:
                                nc.vector.tensor_copy(dst, srcv)
                else:
                    aT_next = aT_p.tile(
                        [128, 6 * NTOK], BF16, tag="at", name=f"at{i + 1}", bufs=1
                    )
                new_rows = [None] * 4
                st6s = [None] * 4
                tdone = []
                for n in range(2):
                    ns = slice(n * 384, (n + 1) * 384)
                    w2ts = []
                    for jc in range(24):
                        wt = w2_p.tile([128, 384], BF16, tag="w2", name=f"w2_{n}_{jc}")
                        nc.sync.dma_start(out=wt[:], in_=d_w2[i, jc * 128 : (jc + 1) * 128, ns])
                        w2ts.append(wt)
                    for t in range(4):
                        ps = mm_ps.tile([128, 512], F32, tag="mm", name=f"f2p{t}{n}")
                        for jc in range(24):
                            nc.tensor.matmul(
                                ps[:, :384],
                                gT[jc][:, t * 128 : (t + 1) * 128],
                                w2ts[jc][:],
                                start=(jc == 0),
                                stop=(jc == 23),
                            )
                        nc.vector.tensor_tensor(h[t][:, ns], h[t][:, ns], ps[:, :384], op=OP.add)
                        if n == 0:
                            st6s[t] = emit_stats0(t, f"a{i + 1}_")
                        elif not last:
                            new_rows[t] = emit_ln_rest(t, f"a{i + 1}_", st6s[t])
                            if t >= 1:
                                transpose_one(new_rows[t - 1], t - 1, aT_next, f"f{i}_")
                        else:
                            hbt = arow_p.tile([128, C], BF16, tag="ar", name=f"hb{t}")
                            nc.scalar.copy(hbt[:], h[t][:])
                            tdone.append((t, hbt))
                            if len(tdone) >= 2:
                                emit_ftr(*tdone[-2])
                if last:
                    emit_ftr(*tdone[-1])
                else:
                    transpose_one(new_rows[2], 2, aT_next, f"f{i}b_")
                    a_rows = new_rows
                    aT = aT_next

            # ---- final: feature-LN stats + head (hT built inline above) ----
            ps_s = mm_ps.tile([128, 512], F32, tag="mm", name="ps_s")
            ps_q = mm_ps.tile([128, 512], F32, tag="mm", name="ps_q")
            for cc in range(6):
                s = aT_p.tile([128, NTOK], BF16, tag="at", name=f"sq{cc}")
                nc.scalar.activation(s[:], hTr(cc), AF.Square)
                nc.tensor.matmul(
                    ps_s[0:1, :], onesB[:], hTr(cc), start=(cc == 0), stop=(cc == 5)
                )
                nc.tensor.matmul(
                    ps_q[0:1, :], onesB[:], s[:], start=(cc == 0), stop=(cc == 5)
                )
            sum_s = sm512_p.tile([1, SP], F32, tag="rbs", name="sum_s", bufs=1)
            nc.vector.tensor_reduce(
                sum_s[:], ps_s[0:1, :].rearrange("p (g s) -> p s g", g=TB),
                axis=AX.X, op=OP.add,
            )
            sum_q = sm512_p.tile([1, SP], F32, tag="rbs", name="sum_q", bufs=1)
            nc.vector.tensor_reduce(
                sum_q[:], ps_q[0:1, :].rearrange("p (g s) -> p s g", g=TB),
                axis=AX.X, op=OP.add,
            )
            mean = sm512_p.tile([1, SP], F32, tag="mn", name="mean")
            nc.vector.tensor_scalar_mul(mean[:], sum_s[:], 1.0 / FD)
            msq = sm512_p.tile([1, SP], F32, tag="mn", name="msq")
            nc.vector.tensor_scalar_mul(msq[:], sum_q[:], 1.0 / FD)
            mm2 = sm512_p.tile([1, SP], F32, tag="rcp", name="mm2")
            nc.vector.tensor_tensor(mm2[:], mean[:], mean[:], op=OP.mult)
            var = sm512_p.tile([1, SP], F32, tag="rcp", name="var")
            nc.vector.tensor_tensor(var[:], msq[:], mm2[:], op=OP.subtract)
            stdf = sm512_p.tile([1, SP], F32, tag="rcp", name="stdf")
            nc.scalar.activation(stdf[:], var[:], AF.Sqrt, bias=eps[0:1, :])
            rstd = sm512_p.tile([1, SP], F32, tag="rcp", name="rstdf")
            nc.vector.reciprocal_approx_fast(out=rstd[:], in_=stdf[:])
            rstdB = sm_p.tile([128, SP], F32, tag="rstdB", name="rstdB", bufs=1)
            nc.gpsimd.partition_broadcast(rstdB[:, 0:SP], rstd[:])
            cm = sm512_p.tile([1, SP], F32, tag="rcp", name="cm")
            nc.vector.tensor_tensor(cm[:], mean[:], rstd[:], op=OP.mult)
            c0 = sm512_p.tile([1, 1], F32, tag="c0", name="c0")
            nc.vector.tensor_reduce(c0[:], cm[:], axis=AX.X, op=OP.add)
            c0B = sm_p.tile([128, 1], F32, tag="c0b", name="c0B")
            nc.gpsimd.partition_broadcast(c0B[:], c0[:])

            ps_l = mm_ps.tile([128, 512], F32, tag="mm", name="ps_l")
            for idx in range(48):
                cc, tb = idx // TB, idx % TB
                g, c = idx // 12, idx % 12
                nc.tensor.matmul(
                    ps_l[0:NCLS, 0:SP],
                    hw[g][:, c * NCLS : (c + 1) * NCLS],
                    hTr(cc)[:, tb * SP : (tb + 1) * SP],
                    start=(idx == 0),
                    stop=(idx == 47),
                )
            gs = sm_p.tile([128, SP], F32, tag="gs", name="gs")
            nc.vector.tensor_tensor(gs[0:NCLS, :], ps_l[0:NCLS, 0:SP], rstdB[0:NCLS, :], op=OP.mult)
            red = sm_p.tile([128, 1], F32, tag="red", name="red")
            nc.vector.tensor_reduce(red[0:NCLS, :], gs[0:NCLS, :], axis=AX.X, op=OP.add)
            t1 = sm_p.tile([128, 1], F32, tag="t1", name="t1")
            nc.vector.tensor_scalar(
                t1[0:NCLS, :],
                cb[0:NCLS, CB_W1 : CB_W1 + 1],
                c0B[0:NCLS, :],
                None,
                op0=OP.mult,
            )
            t2 = sm_p.tile([128, 1], F32, tag="t2", name="t2")
            nc.vector.tensor_tensor(t2[0:NCLS, :], red[0:NCLS, :], t1[0:NCLS, :], op=OP.subtract)
            logits = sm_p.tile([128, 1], F32, tag="lg", name="logits")
            nc.vector.tensor_scalar(
                logits[0:NCLS, :],
                t2[0:NCLS, :],
                1.0 / SP,
                cb[0:NCLS, CB_HB : CB_HB + 1],
                op0=OP.mult,
                op1=OP.add,
            )
            nc.sync.dma_start(out=d_out[:], in_=logits[0:NCLS, :])

    nc.compile()
    return nc


def _prep_inputs(inputs):
    f = np.float32
    x = np.asarray(inputs["x"], f)
    B = x.shape[0]
    xpt = np.empty((B, PVEC, NTOK), bf16)
    for b in range(B):
        xp = x[b, 0].reshape(8, 8, 8, 8, 8, 8).transpose(0, 2, 4, 1, 3, 5).reshape(NTOK, PVEC)
        xpt[b] = np.ascontiguousarray(xp.T).astype(bf16)

    qw, kw, vw, pw = (np.asarray(inputs[k], f) for k in ("qw", "kw", "vw", "pw"))
    f1w, f2w = np.asarray(inputs["f1w"], f), np.asarray(inputs["f2w"], f)
    l1w, l1b = np.asarray(inputs["ln1_w"], f), np.asarray(inputs["ln1_b"], f)
    l2w, l2b = np.asarray(inputs["ln2_w"], f), np.asarray(inputs["ln2_b"], f)

    wq = np.ascontiguousarray((qw * l1w[:, None, :]).transpose(0, 2, 1)).astype(bf16)
    wk = np.ascontiguousarray((kw * l1w[:, None, :]).transpose(0, 2, 1)).astype(bf16)
    wv = np.ascontiguousarray(
        (vw * l1w[:, None, :] * SV).transpose(0, 2, 1)
    ).astype(bf16)
    wp = np.ascontiguousarray(pw.transpose(0, 2, 1)).astype(bf16)
    w1 = np.ascontiguousarray((f1w * l2w[:, None, :]).transpose(0, 2, 1)).astype(bf16)
    w2 = np.ascontiguousarray(f2w.transpose(0, 2, 1)).astype(bf16)

    qb = np.asarray(inputs["qb"], f) + np.einsum("ioc,ic->io", qw, l1b)
    kb = np.asarray(inputs["kb"], f) + np.einsum("ioc,ic->io", kw, l1b)
    vb = (np.asarray(inputs["vb"], f) + np.einsum("ioc,ic->io", vw, l1b)) * SV
    f1b = np.asarray(inputs["f1b"], f) + np.einsum("ijc,ic->ij", f1w, l2b)

    head_w = np.asarray(inputs["head_w"], f)
    fcn_w, fcn_b = np.asarray(inputs["fcn_w"], f), np.asarray(inputs["fcn_b"], f)
    head_b = np.asarray(inputs["head_b"], f) + head_w @ fcn_b
    hwt = np.ascontiguousarray(head_w.T * fcn_w[:, None])
    hwt_b = hwt.astype(bf16)
    # pack the 48 [128, 100] contraction chunks 24-per-tile in (cc, tb)
    # consumption order so the device loads 2 big tiles
    hwt_pk = np.zeros((4, 128, 12 * NCLS), bf16)
    for idx in range(48):
        cc, tb = idx // TB, idx % TB
        row0 = tb * C + cc * 128
        g, c = idx // 12, idx % 12
        hwt_pk[g, :, c * NCLS : (c + 1) * NCLS] = hwt_b[row0 : row0 + 128, :]

    cbp = np.zeros((128, CB_COLS), f)
    for i in range(DEPTH):
        c0 = i * CB_PER_LAYER
        cbp[:, c0 : c0 + 6] = qb[i].reshape(6, 128).T
        cbp[:, c0 + 6 : c0 + 12] = kb[i].reshape(6, 128).T
        cbp[:, c0 + 12 : c0 + 36] = f1b[i].reshape(24, 128).T
    cbp[:NCLS, CB_W1] = hwt_b.astype(f).sum(axis=0)
    cbp[:NCLS, CB_HB] = head_b

    rb = np.stack(
        [
            np.stack(
                [vb[i], np.asarray(inputs["pb"], f)[i], np.asarray(inputs["f2b"], f)[i]]
            )
            for i in range(DEPTH)
        ]
    ).astype(bf16)

    pos2 = (
        np.asarray(inputs["pos_embed"], f)[0] + np.asarray(inputs["patch_b"], f)[None, :]
    ).astype(f)
    pwt = np.ascontiguousarray(np.asarray(inputs["patch_w"], f).T).astype(bf16)

    shared = {
        "pos2": pos2,
        "pwt": pwt,
        "wq": wq,
        "wk": wk,
        "wv": wv,
        "wp": wp,
        "w1": w1,
        "w2": w2,
        "cb": cbp,
        "rb": rb,
        "hwt": hwt_pk,
    }
    return xpt, shared


_NC = None


def _get_nc():
    global _NC
    if _NC is None:
        _NC = _build()
    return _NC


def kernel(**inputs):
    nc = _get_nc()
    xpt, shared = _prep_inputs(inputs)
    B = xpt.shape[0]
    in_maps = [dict(shared, xpt=xpt[b]) for b in range(B)]
    res = run_bass_kernel_spmd(nc, in_maps, list(range(B)))
    return np.stack([res.results[b]["out"] for b in range(B)]).astype(np.float32)


# revision 21
# speedup vs baseline: 1.0575x; 1.0277x over previous
"""HSIViT forward on 8 Trainium2 NeuronCores.

Sharding: pure data parallel — batch B=8, one batch item per core, no
collectives. Each core runs the full 12-layer ViT on its (512, 768)
token activations and emits its (100,) logits row.

Host-side prep (numpy, not counted in HW exec time):
  - patch cubes extracted + transposed per batch item (xpT [512, 512])
  - all weights transposed to [c_in, c_out] for the PE's lhsT layout
  - LN1/LN2 scale+bias folded into q/k/v and fc1 weights+biases
  - v weights+bias pre-scaled by SV so the fp8 eviction needs no extra op
  - final feature-LN scale/bias folded into the classifier head
  - weights cast to bf16; patch embed + head ride bf16 too

Schedule (vs the previous revision):
  - qk projection fused into the attention pipeline per output-column
    tile mc: scores for head pair mc follow qkproj(mc+1), so the ACT
    exp stream starts early and stays hidden behind PE work.
  - score matmuls row-packed: both heads of a pair run concurrently in
    disjoint PE row groups (K=64 each), into the two banks of a
    [128,1024] psum tile; one ACT exp covers both heads' j-chunk.
  - exp output is fp8(e4m3); AV runs fp8 DoubleRow (K=256/pass) with a
    ones-column in V producing the softmax denominator; reciprocal
    batched per head pair; normalization fused into the oT eviction.
  - fc2 (and patch) iterate t in (3,0,1,2) so the last token tile's
    LN chain overlaps the other tiles' matmuls; next layer's aT
    transposes then run stall-free.
  - final head weights prefetched in 2 big DMAs; the feature transpose
    runs inline with fc2 of layer 11; a dummy sqrt warms the ACT table.
"""

import os
import sys

import numpy as np

for _p in ("/opt/trn_rl_repo", "/root/.axon_site/_ro/trn_rl_repo"):
    if _p not in sys.path and os.path.isdir(_p):
        sys.path.insert(0, _p)

import ml_dtypes  # noqa: E402

import concourse.bass as bass  # noqa: E402,F401
import concourse.mybir as mybir  # noqa: E402
import concourse.tile as tile  # noqa: E402
from concourse import bacc  # noqa: E402
from concourse.bass_utils import run_bass_kernel_spmd  # noqa: E402
from concourse.masks import make_identity  # noqa: E402

F32 = mybir.dt.float32
BF16 = mybir.dt.bfloat16
FP8 = mybir.dt.float8e4
AF = mybir.ActivationFunctionType
OP = mybir.AluOpType
AX = mybir.AxisListType
DR = mybir.MatmulPerfMode.DoubleRow

DEPTH, C, NH, HD = 12, 768, 12, 64
NTOK, PVEC = 512, 512
FF = 3072
NCLS = 100
TB, SP = 8, 64
FD = TB * C
SCALE = HD**-0.5
EPS = 1e-5
SV = 32.0  # fp8 scale on the v path (weights+bias pre-scaled host-side)
VSL = NH * (HD + 1) + 4  # 784: per-key-pair-slot v row, padded so 784%16==0

CB_PER_LAYER = 36  # qb 6 + kb 6 + f1b 24 columns
CB_W1 = DEPTH * CB_PER_LAYER
CB_HB = CB_W1 + 1
CB_COLS = CB_HB + 1

bf16 = ml_dtypes.bfloat16


def _build():
    nc = bacc.Bacc(None, target_bir_lowering=False, debug=False)

    d_xpt = nc.dram_tensor("xpt", [PVEC, NTOK], BF16, kind="ExternalInput")
    d_pos2 = nc.dram_tensor("pos2", [NTOK, C], F32, kind="ExternalInput")
    d_pwt = nc.dram_tensor("pwt", [PVEC, C], BF16, kind="ExternalInput")
    d_wq = nc.dram_tensor("wq", [DEPTH, C, C], BF16, kind="ExternalInput")
    d_wk = nc.dram_tensor("wk", [DEPTH, C, C], BF16, kind="ExternalInput")
    d_wv = nc.dram_tensor("wv", [DEPTH, C, C], BF16, kind="ExternalInput")
    d_wp = nc.dram_tensor("wp", [DEPTH, C, C], BF16, kind="ExternalInput")
    d_w1 = nc.dram_tensor("w1", [DEPTH, C, FF], BF16, kind="ExternalInput")
    d_w2 = nc.dram_tensor("w2", [DEPTH, FF, C], BF16, kind="ExternalInput")
    d_cb = nc.dram_tensor("cb", [128, CB_COLS], F32, kind="ExternalInput")
    d_rb = nc.dram_tensor("rb", [DEPTH, 3, C], BF16, kind="ExternalInput")
    d_hwt = nc.dram_tensor("hwt", [4, 128, 12 * NCLS], BF16, kind="ExternalInput")
    d_out = nc.dram_tensor("out", [NCLS], F32, kind="ExternalOutput")

    from contextlib import ExitStack

    with tile.TileContext(nc) as tc:
        with ExitStack() as ctx:
            ep = ctx.enter_context
            const = ep(tc.tile_pool(name="const", bufs=1))
            hpool = ep(tc.tile_pool(name="hpool", bufs=4))
            arow_p = ep(tc.tile_pool(name="arow", bufs=4))
            aT_p = ep(tc.tile_pool(name="atp", bufs=1))
            a2T_p = ep(tc.tile_pool(name="a2tp", bufs=6))
            qT_p = ep(tc.tile_pool(name="qtp", bufs=6))
            kT_p = ep(tc.tile_pool(name="ktp", bufs=6))
            vx_p = ep(tc.tile_pool(name="vxp", bufs=2))
            ex_p = ep(tc.tile_pool(name="exp", bufs=3))
            oT_p = ep(tc.tile_pool(name="otp", bufs=6))
            gT_p = ep(tc.tile_pool(name="gtp", bufs=24))
            qkw_p = ep(tc.tile_pool(name="qkw", bufs=12))
            vpw_p = ep(tc.tile_pool(name="vpw", bufs=9))
            w1_p = ep(tc.tile_pool(name="w1p", bufs=12))
            w2_p = ep(tc.tile_pool(name="w2p", bufs=9))
            hw_p = ep(tc.tile_pool(name="hwp", bufs=2))
            bc_p = ep(tc.tile_pool(name="bcp", bufs=2))
            rcp_p = ep(tc.tile_pool(name="rcpp", bufs=2))
            den_p = ep(tc.tile_pool(name="denp", bufs=1))
            sm_p = ep(tc.tile_pool(name="smp", bufs=8))
            sm512_p = ep(tc.tile_pool(name="sm512", bufs=2))
            mm_ps = ep(tc.tile_pool(name="mmps", bufs=3, space="PSUM"))
            sc_ps = ep(tc.tile_pool(name="scps", bufs=2, space="PSUM"))
            tp_ps = ep(tc.tile_pool(name="tpps", bufs=1, space="PSUM"))

            ident = const.tile([128, 128], F32, tag="ident", name="ident")
            make_identity(nc, ident)
            identB = const.tile([128, 128], BF16, tag="identB", name="identB")
            nc.scalar.copy(identB[:], ident[:])
            ones0 = const.tile([128, 1], F32, tag="ones0", name="ones0")
            nc.vector.memset(ones0[:], 1.0)
            onesB = const.tile([128, 1], BF16, tag="onesB", name="onesB")
            nc.scalar.copy(onesB[:], ones0[:])
            ones_r = const.tile([1, 64], F32, tag="ones_r", name="ones_r")
            nc.vector.memset(ones_r[:], 1.0)
            eps = const.tile([128, 1], F32, tag="eps", name="eps")
            nc.vector.memset(eps[:], EPS)
            cb = const.tile([128, CB_COLS], F32, tag="cb", name="cb")
            nc.sync.dma_start(out=cb[:], in_=d_cb[:])

            h = []
            for t in range(4):
                ht = hpool.tile([128, C], F32, tag="h", name=f"h{t}")
                h.append(ht)

            def emit_stats0(t, tag):
                st6 = sm_p.tile([128, 12], F32, tag="st6", name=f"st6_{tag}{t}")
                nc.vector.bn_stats(st6[:, 0:6], h[t][:, 0:384])
                return st6

            def emit_ln_rest(t, tag, st6):
                nc.vector.bn_stats(st6[:, 6:12], h[t][:, 384:768])
                mv = sm_p.tile([128, 2], F32, tag="mv", name=f"mv{tag}{t}")
                nc.vector.bn_aggr(mv[:], st6.rearrange("p (g s) -> p g s", g=2))
                std = sm_p.tile([128, 1], F32, tag="std", name=f"std{tag}{t}")
                nc.scalar.activation(std[:], mv[:, 1:2], AF.Sqrt, bias=eps[:])
                rstd = sm_p.tile([128, 1], F32, tag="rstd", name=f"rstd{tag}{t}")
                nc.vector.reciprocal_approx_fast(out=rstd[:], in_=std[:])
                at = arow_p.tile([128, C], BF16, tag="ar", name=f"ar{tag}{t}")
                nc.vector.tensor_scalar(
                    at[:], h[t], mv[:, 0:1], rstd[:], op0=OP.subtract, op1=OP.mult
                )
                return at

            def transpose_pass(rows, t_list, col0, outs, tag2):
                """Transpose token tiles t_list into cols [col0:] of the 6
                col tiles; evictions alternate ACT/DVE on tp-tile halves."""
                w = 128 * len(t_list)
                tp = tp_ps.tile([128, 1024], BF16, tag="tp", name=f"tp{tag2}")
                for cc in range(6):
                    sl = tp[:, (cc % 2) * 512 : (cc % 2) * 512 + 512]
                    for ti, t in enumerate(t_list):
                        nc.tensor.transpose(
                            sl[:, ti * 128 : (ti + 1) * 128],
                            rows[t][:, cc * 128 : (cc + 1) * 128],
                            identB[:],
                        )
                    dst = outs[cc][:, col0 : col0 + w]
                    if cc % 2 == 0:
                        nc.scalar.copy(dst, sl[:, 0:w])
                    else:
                        nc.vector.tensor_copy(dst, sl[:, 0:w])

            def transpose_big(rows, order, big, tag2):
                """Per token tile: 6 PE transposes + 2 strided evictions into
                the [128, 6*512] column-major tile; tp halves double-buffer."""
                bigr = big.rearrange("p (c w) -> p c w", c=6)
                tp = tp_ps.tile([128, 1024], BF16, tag="tp", name=f"tp{tag2}")
                k = 0
                for t in order:
                    for c0, ncc in ((0, 4), (4, 2)):
                        sl = tp[:, (k % 2) * 512 : (k % 2) * 512 + 128 * ncc]
                        for ci in range(ncc):
                            nc.tensor.transpose(
                                sl[:, ci * 128 : (ci + 1) * 128],
                                rows[t][:, (c0 + ci) * 128 : (c0 + ci + 1) * 128],
                                identB[:],
                            )
                        dst = bigr[:, c0 : c0 + ncc, t * 128 : (t + 1) * 128]
                        srcv = sl.rearrange("p (c w) -> p c w", c=ncc)
                        if k % 2 == 0:
                            nc.scalar.copy(dst, srcv)
                        else:
                            nc.vector.tensor_copy(dst, srcv)
                        k += 1

            def bcast_row(i, j, tag):
                src = sm512_p.tile([1, C], BF16, tag="rbs", name=f"rbs{i}_{j}", bufs=1)
                nc.sync.dma_start(out=src[:], in_=d_rb[i, j])
                bt = bc_p.tile([128, C], BF16, tag="bc", name=f"{tag}{i}")
                nc.gpsimd.partition_broadcast(bt[:], src[:])
                return bt

            # persistent transpose psum tile; halves double-buffer globally
            tpb = tp_ps.tile([128, 1024], BF16, tag="tp", name="tpb")
            tp_k = [0]

            def transpose_one(row_t, t, big, tag2, eng="alt"):
                """6 PE transposes of one token tile into the [128, 6*512]
                column-major tile; 2 strided evictions (ACT, DVE, or both)."""
                bigr = big.rearrange("p (c w) -> p c w", c=6)
                for c0, ncc in ((0, 4), (4, 2)):
                    k = tp_k[0]
                    tp_k[0] += 1
                    sl = tpb[:, (k % 2) * 512 : (k % 2) * 512 + 128 * ncc]
                    for ci in range(ncc):
                        nc.tensor.transpose(
                            sl[:, ci * 128 : (ci + 1) * 128],
                            row_t[:, (c0 + ci) * 128 : (c0 + ci + 1) * 128],
                            identB[:],
                        )
                    dst = bigr[:, c0 : c0 + ncc, t * 128 : (t + 1) * 128]
                    srcv = sl.rearrange("p (c w) -> p c w", c=ncc)
                    on_act = (eng == "act") or (eng == "alt" and k % 2 == 0)
                    if on_act:
                        nc.scalar.copy(dst, srcv)
                    else:
                        nc.vector.tensor_copy(dst, srcv)

            # ---- patch embed: h = pos(+patch_b) + xp @ patch_w.T ----
            # lag-1 LN + transposes into layer 0's aT; t3 left pending.
            xpt = []
            pwt = []
            for kc in range(4):
                xt = qT_p.tile([128, NTOK], BF16, tag="qt", name=f"xpt{kc}")
                nc.sync.dma_start(out=xt[:], in_=d_xpt[kc * 128 : (kc + 1) * 128, :])
                xpt.append(xt)
            for kc in range(4):
                wt = vpw_p.tile([128, C], BF16, tag="vpw", name=f"pwt{kc}")
                nc.sync.dma_start(out=wt[:], in_=d_pwt[kc * 128 : (kc + 1) * 128, :])
                pwt.append(wt)
            a_rows = [None] * 4
            aT = aT_p.tile([128, 6 * NTOK], BF16, tag="at", name="at_l0", bufs=1)
            for t in range(4):
                nc.sync.dma_start(out=h[t][:], in_=d_pos2[t * 128 : (t + 1) * 128, :])
                st6 = None
                for n in range(2):
                    ns = slice(n * 384, (n + 1) * 384)
                    ps = mm_ps.tile([128, 512], F32, tag="mm", name=f"pep{t}{n}")
                    for kc in range(4):
                        nc.tensor.matmul(
                            ps[:, :384],
                            xpt[kc][:, t * 128 : (t + 1) * 128],
                            pwt[kc][:, ns],
                            start=(kc == 0),
                            stop=(kc == 3),
                        )
                    nc.vector.tensor_tensor(h[t][:, ns], h[t][:, ns], ps[:, :384], op=OP.add)
                    if n == 0:
                        st6 = emit_stats0(t, "a0_")
                a_rows[t] = emit_ln_rest(t, "a0_", st6)
                if t >= 1:
                    transpose_one(a_rows[t - 1], t - 1, aT, "pa")
            transpose_one(a_rows[2], 2, aT, "pb")
            # a_rows[3] transpose pending; done at layer-0 boundary

            for i in range(DEPTH):
                cb0 = i * CB_PER_LAYER
                # ---- weights for this layer ----
                qk_w = []
                for (dw, tg) in ((d_wq, "qw"), (d_wk, "kw")):
                    wts = []
                    for kc in range(6):
                        wt = qkw_p.tile([128, C], BF16, tag="qkw", name=f"{tg}{kc}")
                        nc.sync.dma_start(out=wt[:], in_=dw[i, kc * 128 : (kc + 1) * 128, :])
                        wts.append(wt)
                    qk_w.append(wts)
                vwts = []
                for kc in range(6):
                    wt = vpw_p.tile([128, C], BF16, tag="vpw", name=f"vw{kc}")
                    nc.sync.dma_start(out=wt[:], in_=d_wv[i, kc * 128 : (kc + 1) * 128, :])
                    vwts.append(wt)
                pwts = []
                for kc in range(6):
                    wt = vpw_p.tile([128, C], BF16, tag="vpw", name=f"pw{kc}")
                    nc.sync.dma_start(out=wt[:], in_=d_wp[i, kc * 128 : (kc + 1) * 128, :])
                    pwts.append(wt)

                vbB = bcast_row(i, 0, "vb")
                pbB = bcast_row(i, 1, "pb")

                vx = []
                for g in range(2):
                    vt = vx_p.tile([128, 2 * VSL], FP8, tag="vx", name=f"vx{g}")
                    vx.append(vt)
                    for s in range(2):
                        ones_sl = vt[:, s * VSL : s * VSL + NH * 65].rearrange(
                            "p (h d) -> p h d", h=NH
                        )[:, :, HD : HD + 1]
                        nc.vector.memset(ones_sl, 1.0)

                qT = [None] * 6
                kT = [None] * 6
                ex_all = [None] * 6
                oT = []
                for cc in range(6):
                    ot = oT_p.tile([128, NTOK], BF16, tag="ot", name=f"ot{cc}")
                    oT.append(ot)

                def qkproj0a(which):
                    wts, base = (qk_w[0], cb0) if which == 0 else (qk_w[1], cb0 + 6)
                    out = (qT_p if which == 0 else kT_p).tile(
                        [128, NTOK], BF16, tag="qt" if which == 0 else "kt",
                        name=f"{'qk'[which]}T0",
                    )
                    ps = mm_ps.tile([128, 512], F32, tag="mm", name=f"qk0a_{which}")
                    for k in range(6):
                        nc.tensor.matmul(
                            ps[:, 0:384],
                            wts[k][:, 0:128],
                            aT[:, k * 512 : k * 512 + 384],
                            start=(k == 0),
                            stop=(k == 5),
                        )
                    if which == 0:
                        nc.scalar.activation(
                            out[:, 0:384], ps[:, 0:384], AF.Identity, bias=cb[:, base : base + 1]
                        )
                    else:
                        nc.vector.tensor_scalar_add(
                            out[:, 0:384], ps[:, 0:384], cb[:, base : base + 1]
                        )
                    return out

                def qkproj0b(which, out):
                    wts, base = (qk_w[0], cb0) if which == 0 else (qk_w[1], cb0 + 6)
                    ps = mm_ps.tile([128, 512], F32, tag="mm", name=f"qk0b_{which}")
                    for k in range(6):
                        nc.tensor.matmul(
                            ps[:, 0:128],
                            wts[k][:, 0:128],
                            aT[:, k * 512 + 384 : k * 512 + 512],
                            start=(k == 0),
                            stop=(k == 5),
                        )
                    if which == 0:
                        nc.scalar.activation(
                            out[:, 384:512], ps[:, 0:128], AF.Identity, bias=cb[:, base : base + 1]
                        )
                    else:
                        nc.vector.tensor_scalar_add(
                            out[:, 384:512], ps[:, 0:128], cb[:, base : base + 1]
                        )

                def emit_qkproj_one(mc, which):
                    wts, outs, base = (
                        (qk_w[0], qT, cb0) if which == 0 else (qk_w[1], kT, cb0 + 6)
                    )
                    ps = mm_ps.tile([128, 512], F32, tag="mm", name=f"qkp{mc}_{which}")
                    for k in range(6):
                        kc = (k + mc) % 6
                        nc.tensor.matmul(
                            ps[:],
                            wts[kc][:, mc * 128 : (mc + 1) * 128],
                            aT[:, kc * 512 : (kc + 1) * 512],
                            start=(k == 0),
                            stop=(k == 5),
                        )
                    out = (qT_p if which == 0 else kT_p).tile(
                        [128, NTOK], BF16, tag="qt" if which == 0 else "kt",
                        name=f"{'qk'[which]}T{mc}",
                    )
                    if which == 0:
                        nc.scalar.activation(
                            out[:], ps[:], AF.Identity, bias=cb[:, base + mc : base + mc + 1]
                        )
                    else:
                        nc.vector.tensor_scalar_add(
                            out[:], ps[:], cb[:, base + mc : base + mc + 1]
                        )
                    outs[mc] = out

                def emit_pair_scores(p, jlist, expair):
                    for j in jlist:
                        sc = sc_ps.tile([128, 1024], F32, tag="sc", name=f"sc{p}_{j}")
                        for hi in range(2):
                            off = hi * 64
                            nc.tensor.matmul(
                                sc[:, hi * 512 : (hi + 1) * 512],
                                kT[p][off : off + 64, j * 128 : (j + 1) * 128],
                                qT[p][off : off + 64, :],
                                start=True,
                                stop=True,
                            )
                        nc.scalar.activation(
                            expair[:, j * 1024 : (j + 1) * 1024], sc[:], AF.Exp,
                            scale=SCALE,
                        )
                    ex_all[p] = expair

                def emit_vgroup(t):
                    vxt = vx[t // 2]
                    base = (t % 2) * VSL
                    for n in range(2):
                        ps = mm_ps.tile([128, 512], F32, tag="mm", name=f"vp{t}{n}")
                        for kc in range(6):
                            nc.tensor.matmul(
                                ps[:, :384],
                                aT[:, kc * 512 + t * 128 : kc * 512 + (t + 1) * 128],
                                vwts[kc][:, n * 384 : (n + 1) * 384],
                                start=(kc == 0),
                                stop=(kc == 5),
                            )
                        dst = vxt[:, base + n * 6 * 65 : base + (n + 1) * 6 * 65].rearrange(
                            "p (h d) -> p h d", h=6
                        )[:, :, 0:HD]
                        nc.vector.tensor_tensor(
                            dst,
                            ps[:, :384].rearrange("p (g d) -> p g d", g=6),
                            vbB[:, n * 384 : (n + 1) * 384].rearrange(
                                "p (g d) -> p g d", g=6
                            ),
                            op=OP.add,
                        )

                def emit_pair_av(p):
                    exr = ex_all[p].rearrange("q (j c) -> q j c", j=4)
                    pos_ = []
                    rcps = []
                    for hi in range(2):
                        hh = 2 * p + hi
                        po = mm_ps.tile([128, 512], F32, tag="mm", name=f"po{hh}")
                        for jp in range(2):
                            nc.tensor.matmul(
                                po[0 : HD + 1, :],
                                vx[jp].rearrange("q (s c) -> q s c", s=2)[
                                    :, :, hh * 65 : hh * 65 + 65
                                ],
                                exr[:, 2 * jp : 2 * jp + 2, hi * 512 : (hi + 1) * 512],
                                start=(jp == 0),
                                stop=(jp == 1),
                                perf_mode=DR,
                            )
                        den = den_p.tile([1, NTOK], F32, tag="dr", name=f"den{hh}", bufs=2)
                        nc.vector.tensor_scalar_mul(den[:], po[HD : HD + 1, :], SV)
                        rcp = den_p.tile([1, NTOK], F32, tag="dr", name=f"rcp{hh}", bufs=2)
                        nc.vector.reciprocal_approx_fast(out=rcp[:], in_=den[:])
                        rb_ = rcp_p.tile([64, NTOK], F32, tag="rb", name=f"rcpB{hh}")
                        nc.gpsimd.partition_broadcast(rb_[:], rcp[:])
                        pos_.append(po)
                        rcps.append(rb_)
                    for hi in range(2):
                        hh = 2 * p + hi
                        nc.vector.tensor_tensor(
                            oT[hh // 2][(hh % 2) * 64 : (hh % 2) * 64 + 64, :],
                            pos_[hi][0:HD, :],
                            rcps[hi][:],
                            op=OP.mult,
                        )

                # ---- boundary: split qk-proj for mc=0 around the pending
                # t3 transpose, then the mc pipeline ----
                out_q0 = qkproj0a(0)
                out_k0 = qkproj0a(1)
                transpose_one(a_rows[3], 3, aT, f"bd{i}")
                qkproj0b(0, out_q0)
                qkproj0b(1, out_k0)
                qT[0] = out_q0
                kT[0] = out_k0

                for mc in range(1, 6):
                    emit_qkproj_one(mc, 0)
                    ex_t = ex_p.tile([128, 4096], FP8, tag="ex", name=f"ex{mc - 1}")
                    emit_pair_scores(mc - 1, (0, 1), ex_t)
                    emit_qkproj_one(mc, 1)
                    emit_pair_scores(mc - 1, (2, 3), ex_t)
                    if mc == 1:
                        emit_vgroup(0)
                        emit_vgroup(1)
                    if mc == 2:
                        emit_vgroup(2)
                        emit_vgroup(3)
                        nc.vector.tensor_tensor(h[0][:], h[0][:], pbB[:], op=OP.add)
                        nc.vector.tensor_tensor(h[1][:], h[1][:], pbB[:], op=OP.add)
                    if mc == 3:
                        nc.vector.tensor_tensor(h[2][:], h[2][:], pbB[:], op=OP.add)
                        nc.vector.tensor_tensor(h[3][:], h[3][:], pbB[:], op=OP.add)
                        emit_pair_av(0)
                    if mc == 4:
                        emit_pair_av(1)
                        emit_pair_av(2)
                    if mc == 5:
                        emit_pair_av(3)
                ex_t = ex_p.tile([128, 4096], FP8, tag="ex", name="ex5")
                emit_pair_scores(5, (0, 1), ex_t)
                emit_pair_scores(5, (2, 3), ex_t)
                emit_pair_av(4)
                emit_pair_av(5)

                # ---- output projection + residual (pb pre-added), lag-1 LN2
                # transposes into a2T; t3 pending into fc1 ----
                a2T = a2T_p.tile([128, 6 * NTOK], BF16, tag="a2t", name=f"a2t{i}", bufs=1)
                a2_rows = [None] * 4
                for t in range(4):
                    st6 = None
                    for n in range(2):
                        ns = slice(n * 384, (n + 1) * 384)
                        ps = mm_ps.tile([128, 512], F32, tag="mm", name=f"prj{t}{n}")
                        for k in range(6):
                            nc.tensor.matmul(
                                ps[:, :384],
                                oT[k][:, t * 128 : (t + 1) * 128],
                                pwts[k][:, ns],
                                start=(k == 0),
                                stop=(k == 5),
                            )
                        nc.vector.tensor_tensor(h[t][:, ns], h[t][:, ns], ps[:, :384], op=OP.add)
                        if n == 0:
                            st6 = emit_stats0(t, f"b{i}_")
                    a2_rows[t] = emit_ln_rest(t, f"b{i}_", st6)
                    if t >= 1:
                        transpose_one(a2_rows[t - 1], t - 1, a2T, f"p{i}_", eng="act")
                transpose_one(a2_rows[2], 2, a2T, f"p{i}b_", eng="act")

                # ---- fc1 + gelu; the 0:384 pass hides t3's LN + transpose ----
                f2bB = bcast_row(i, 2, "fb")
                gT = [gT_p.tile([128, NTOK], BF16, tag="gt", name=f"gt{m}") for m in range(24)]
                for half in range(2):
                    wts = []
                    for kc in range(6):
                        wt = w1_p.tile([128, FF // 2], BF16, tag="w1", name=f"w1_{half}_{kc}")
                        nc.sync.dma_start(
                            out=wt[:],
                            in_=d_w1[
                                i,
                                kc * 128 : (kc + 1) * 128,
                                half * (FF // 2) : (half + 1) * (FF // 2),
                            ],
                        )
                        wts.append(wt)
                    passes = ((0, 384), (384, 512)) if half == 0 else ((0, 512),)
                    for cs, ce in passes:
                        w = ce - cs
                        for mh in range(12):
                            m = half * 12 + mh
                            ps = mm_ps.tile([128, 512], F32, tag="mm", name=f"f1p{m}")
                            for k in range(6):
                                kc = (k + mh) % 6
                                nc.tensor.matmul(
                                    ps[:, 0:w],
                                    wts[kc][:, mh * 128 : (mh + 1) * 128],
                                    a2T[:, kc * 512 + cs : kc * 512 + ce],
                                    start=(k == 0),
                                    stop=(k == 5),
                                )
                            nc.scalar.activation(
                                gT[m][:, cs:ce], ps[:, 0:w], AF.Gelu,
                                bias=cb[:, cb0 + 12 + m : cb0 + 13 + m],
                            )
                            if half == 0 and cs == 0 and mh == 2:
                                transpose_one(a2_rows[3], 3, a2T, f"p{i}c_", eng="act")
                        if half == 0 and cs == 0:
                            for t in range(4):
                                nc.vector.tensor_tensor(h[t][:], h[t][:], f2bB[:], op=OP.add)

                # ---- fc2 + residual; lag-1 next-layer LN1 + aT transposes
                # (or the final feature transpose on the last layer) ----
                last = i == DEPTH - 1
                if last:
                    hw = []
                    for g in range(4):
                        hwt_t = hw_p.tile([128, 12 * NCLS], BF16, tag="hw", name=f"hw{g}")
                        nc.sync.dma_start(out=hwt_t[:], in_=d_hwt[g])
                        hw.append(hwt_t)
                    dum = sm_p.tile([1, 1], F32, tag="dum", name="dum")
                    nc.scalar.activation(dum[:], eps[0:1, :], AF.Sqrt)
                    hTa = w1_p.tile([128, 3 * NTOK], BF16, tag="w1", name="hTa")
                    hTb = w1_p.tile([128, 3 * NTOK], BF16, tag="w1", name="hTb")

                    def hTr(cc):
                        t_ = hTa if cc < 3 else hTb
                        return t_.rearrange("p (c w) -> p c w", c=3)[:, cc % 3, :]

                    def emit_ftr(t, hbt):
                        for c0, dtile in ((0, hTa), (3, hTb)):
                            k = tp_k[0]
                            tp_k[0] += 1
                            sl = tpb[:, (k % 2) * 512 : (k % 2) * 512 + 384]
                            for ci in range(3):
                                nc.tensor.transpose(
                                    sl[:, ci * 128 : (ci + 1) * 128],
                                    hbt[:, (c0 + ci) * 128 : (c0 + ci + 1) * 128],
                                    identB[:],
                                )
                            dst = dtile.rearrange("p (c w) -> p c w", c=3)[
                                :, :, t * 128 : (t + 1) * 128
                            ]
                            srcv = sl.rearrange("p (c w) -> p c w", c=3)
                            if k % 2 == 0:
                                nc.scalar.copy(dst, srcv)
                            else:
                                nc.vector.tensor_copy(dst, srcv)
                else:
                    aT_next = aT_p.tile(
                        [128, 6 * NTOK], BF16, tag="at", name=f"at{i + 1}", bufs=1
                    )
                new_rows = [None] * 4
                st6s = [None] * 4
                tdone = []
                for n in range(2):
                    ns = slice(n * 384, (n + 1) * 384)
                    w2ts = []
                    for jq in range(6):
                        wt = w2_p.tile([128, 4 * 384], BF16, tag="w2", name=f"w2_{n}_{jq}")
                        nc.sync.dma_start(
                            out=wt.rearrange("p (c w) -> p c w", c=4),
                            in_=d_w2[i, jq * 512 : (jq + 1) * 512, ns].rearrange(
                                "(c p) w -> p c w", c=4
                            ),
                        )
                        w2ts.append(wt)
                    for t in range(4):
                        ps = mm_ps.tile([128, 512], F32, tag="mm", name=f"f2p{t}{n}")
                        for jc in range(24):
                            nc.tensor.matmul(
                                ps[:, :384],
                                gT[jc][:, t * 128 : (t + 1) * 128],
                                w2ts[jc // 4][:, (jc % 4) * 384 : (jc % 4 + 1) * 384],
                                start=(jc == 0),
                                stop=(jc == 23),
                            )
                        nc.vector.tensor_tensor(h[t][:, ns], h[t][:, ns], ps[:, :384], op=OP.add)
                        if n == 0:
                            st6s[t] = emit_stats0(t, f"a{i + 1}_")
                        elif not last:
                            new_rows[t] = emit_ln_rest(t, f"a{i + 1}_", st6s[t])
                            if t >= 1:
                                transpose_one(new_rows[t - 1], t - 1, aT_next, f"f{i}_", eng="act")
                        else:
                            hbt = arow_p.tile([128, C], BF16, tag="ar", name=f"hb{t}")
                            nc.scalar.copy(hbt[:], h[t][:])
                            tdone.append((t, hbt))
                            if len(tdone) >= 2:
                                emit_ftr(*tdone[-2])
                if last:
                    emit_ftr(*tdone[-1])
                else:
                    transpose_one(new_rows[2], 2, aT_next, f"f{i}b_", eng="act")
                    a_rows = new_rows
                    aT = aT_next

            # ---- final: feature-LN stats + head (hT built inline above) ----
            ps_s = mm_ps.tile([128, 512], F32, tag="mm", name="ps_s")
            ps_q = mm_ps.tile([128, 512], F32, tag="mm", name="ps_q")
            for cc in range(6):
                s = aT_p.tile([128, NTOK], BF16, tag="at", name=f"sq{cc}")
                nc.scalar.activation(s[:], hTr(cc), AF.Square)
                nc.tensor.matmul(
                    ps_s[0:1, :], onesB[:], hTr(cc), start=(cc == 0), stop=(cc == 5)
                )
                nc.tensor.matmul(
                    ps_q[0:1, :], onesB[:], s[:], start=(cc == 0), stop=(cc == 5)
                )
            sum_s = sm512_p.tile([1, SP], F32, tag="rbs", name="sum_s", bufs=1)
            nc.vector.tensor_reduce(
                sum_s[:], ps_s[0:1, :].rearrange("p (g s) -> p s g", g=TB),
                axis=AX.X, op=OP.add,
            )
            sum_q = sm512_p.tile([1, SP], F32, tag="rbs", name="sum_q", bufs=1)
            nc.vector.tensor_reduce(
                sum_q[:], ps_q[0:1, :].rearrange("p (g s) -> p s g", g=TB),
                axis=AX.X, op=OP.add,
            )
            mean = sm512_p.tile([1, SP], F32, tag="mn", name="mean")
            nc.vector.tensor_scalar_mul(mean[:], sum_s[:], 1.0 / FD)
            msq = sm512_p.tile([1, SP], F32, tag="mn", name="msq")
            nc.vector.tensor_scalar_mul(msq[:], sum_q[:], 1.0 / FD)
            mm2 = sm512_p.tile([1, SP], F32, tag="rcp", name="mm2")
            nc.vector.tensor_tensor(mm2[:], mean[:], mean[:], op=OP.mult)
            var = sm512_p.tile([1, SP], F32, tag="rcp", name="var")
            nc.vector.tensor_tensor(var[:], msq[:], mm2[:], op=OP.subtract)
            stdf = sm512_p.tile([1, SP], F32, tag="rcp", name="stdf")
            nc.scalar.activation(stdf[:], var[:], AF.Sqrt, bias=eps[0:1, :])
            rstd = sm512_p.tile([1, SP], F32, tag="rcp", name="rstdf")
            nc.vector.reciprocal_approx_fast(out=rstd[:], in_=stdf[:])
            rstdB = sm_p.tile([128, SP], F32, tag="rstdB", name="rstdB", bufs=1)
            nc.gpsimd.partition_broadcast(rstdB[:, 0:SP], rstd[:])
            cm = sm512_p.tile([1, SP], F32, tag="rcp", name="cm")
            nc.vector.tensor_tensor(cm[:], mean[:], rstd[:], op=OP.mult)
            c0 = sm512_p.tile([1, 1], F32, tag="c0", name="c0")
            nc.vector.tensor_reduce(c0[:], cm[:], axis=AX.X, op=OP.add)
            c0B = sm_p.tile([128, 1], F32, tag="c0b", name="c0B")
            nc.gpsimd.partition_broadcast(c0B[:], c0[:])

            ps_l = mm_ps.tile([128, 512], F32, tag="mm", name="ps_l")
            for idx in range(48):
                cc, tb = idx // TB, idx % TB
                g, c = idx // 12, idx % 12
                nc.tensor.matmul(
                    ps_l[0:NCLS, 0:SP],
                    hw[g][:, c * NCLS : (c + 1) * NCLS],
                    hTr(cc)[:, tb * SP : (tb + 1) * SP],
                    start=(idx == 0),
                    stop=(idx == 47),
                )
            gs = sm_p.tile([128, SP], F32, tag="gs", name="gs", bufs=1)
            nc.vector.tensor_tensor(gs[0:NCLS, :], ps_l[0:NCLS, 0:SP], rstdB[0:NCLS, :], op=OP.mult)
            red = sm_p.tile([128, 1], F32, tag="red", name="red", bufs=1)
            nc.vector.tensor_reduce(red[0:NCLS, :], gs[0:NCLS, :], axis=AX.X, op=OP.add)
            t1 = sm_p.tile([128, 1], F32, tag="t1", name="t1")
            nc.vector.tensor_scalar(
                t1[0:NCLS, :],
                cb[0:NCLS, CB_W1 : CB_W1 + 1],
                c0B[0:NCLS, :],
                None,
                op0=OP.mult,
            )
            t2 = sm_p.tile([128, 1], F32, tag="t2", name="t2")
            nc.vector.tensor_tensor(t2[0:NCLS, :], red[0:NCLS, :], t1[0:NCLS, :], op=OP.subtract)
            logits = sm_p.tile([128, 1], F32, tag="lg", name="logits")
            nc.vector.tensor_scalar(
                logits[0:NCLS, :],
                t2[0:NCLS, :],
                1.0 / SP,
                cb[0:NCLS, CB_HB : CB_HB + 1],
                op0=OP.mult,
                op1=OP.add,
            )
            nc.sync.dma_start(out=d_out[:], in_=logits[0:NCLS, :])

    nc.compile()
    return nc


def _prep_inputs(inputs):
    f = np.float32
    x = np.asarray(inputs["x"], f)
    B = x.shape[0]
    xpt = np.empty((B, PVEC, NTOK), bf16)
    for b in range(B):
        xp = x[b, 0].reshape(8, 8, 8, 8, 8, 8).transpose(0, 2, 4, 1, 3, 5).reshape(NTOK, PVEC)
        xpt[b] = np.ascontiguousarray(xp.T).astype(bf16)

    qw, kw, vw, pw = (np.asarray(inputs[k], f) for k in ("qw", "kw", "vw", "pw"))
    f1w, f2w = np.asarray(inputs["f1w"], f), np.asarray(inputs["f2w"], f)
    l1w, l1b = np.asarray(inputs["ln1_w"], f), np.asarray(inputs["ln1_b"], f)
    l2w, l2b = np.asarray(inputs["ln2_w"], f), np.asarray(inputs["ln2_b"], f)

    wq = np.ascontiguousarray((qw * l1w[:, None, :]).transpose(0, 2, 1)).astype(bf16)
    wk = np.ascontiguousarray((kw * l1w[:, None, :]).transpose(0, 2, 1)).astype(bf16)
    wv = np.ascontiguousarray(
        (vw * l1w[:, None, :] * SV).transpose(0, 2, 1)
    ).astype(bf16)
    wp = np.ascontiguousarray(pw.transpose(0, 2, 1)).astype(bf16)
    w1 = np.ascontiguousarray((f1w * l2w[:, None, :]).transpose(0, 2, 1)).astype(bf16)
    w2 = np.ascontiguousarray(f2w.transpose(0, 2, 1)).astype(bf16)

    qb = np.asarray(inputs["qb"], f) + np.einsum("ioc,ic->io", qw, l1b)
    kb = np.asarray(inputs["kb"], f) + np.einsum("ioc,ic->io", kw, l1b)
    vb = (np.asarray(inputs["vb"], f) + np.einsum("ioc,ic->io", vw, l1b)) * SV
    f1b = np.asarray(inputs["f1b"], f) + np.einsum("ijc,ic->ij", f1w, l2b)

    head_w = np.asarray(inputs["head_w"], f)
    fcn_w, fcn_b = np.asarray(inputs["fcn_w"], f), np.asarray(inputs["fcn_b"], f)
    head_b = np.asarray(inputs["head_b"], f) + head_w @ fcn_b
    hwt = np.ascontiguousarray(head_w.T * fcn_w[:, None])
    hwt_b = hwt.astype(bf16)
    # pack the 48 [128, 100] contraction chunks 24-per-tile in (cc, tb)
    # consumption order so the device loads 2 big tiles
    hwt_pk = np.zeros((4, 128, 12 * NCLS), bf16)
    for idx in range(48):
        cc, tb = idx // TB, idx % TB
        row0 = tb * C + cc * 128
        g, c = idx // 12, idx % 12
        hwt_pk[g, :, c * NCLS : (c + 1) * NCLS] = hwt_b[row0 : row0 + 128, :]

    cbp = np.zeros((128, CB_COLS), f)
    for i in range(DEPTH):
        c0 = i * CB_PER_LAYER
        cbp[:, c0 : c0 + 6] = qb[i].reshape(6, 128).T
        cbp[:, c0 + 6 : c0 + 12] = kb[i].reshape(6, 128).T
        cbp[:, c0 + 12 : c0 + 36] = f1b[i].reshape(24, 128).T
    cbp[:NCLS, CB_W1] = hwt_b.astype(f).sum(axis=0)
    cbp[:NCLS, CB_HB] = head_b

    rb = np.stack(
        [
            np.stack(
                [vb[i], np.asarray(inputs["pb"], f)[i], np.asarray(inputs["f2b"], f)[i]]
            )
            for i in range(DEPTH)
        ]
    ).astype(bf16)

    pos2 = (
        np.asarray(inputs["pos_embed"], f)[0] + np.asarray(inputs["patch_b"], f)[None, :]
    ).astype(f)
    pwt = np.ascontiguousarray(np.asarray(inputs["patch_w"], f).T).astype(bf16)

    shared = {
        "pos2": pos2,
        "pwt": pwt,
        "wq": wq,
        "wk": wk,
        "wv": wv,
        "wp": wp,
        "w1": w1,
        "w2": w2,
        "cb": cbp,
        "rb": rb,
        "hwt": hwt_pk,
    }
    return xpt, shared


_NC = None


def _get_nc():
    global _NC
    if _NC is None:
        _NC = _build()
    return _NC


def kernel(**inputs):
    nc = _get_nc()
    xpt, shared = _prep_inputs(inputs)
    B = xpt.shape[0]
    in_maps = [dict(shared, xpt=xpt[b]) for b in range(B)]
    res = run_bass_kernel_spmd(nc, in_maps, list(range(B)))
    return np.stack([res.results[b]["out"] for b in range(B)]).astype(np.float32)


# revision 23
# speedup vs baseline: 1.0759x; 1.0174x over previous
"""HSIViT forward on 8 Trainium2 NeuronCores.

Sharding: pure data parallel — batch B=8, one batch item per core, no
collectives. Each core runs the full 12-layer ViT on its (512, 768)
token activations and emits its (100,) logits row.

Host-side prep (numpy, not counted in HW exec time):
  - patch cubes extracted + transposed per batch item (xpT [512, 512])
  - all weights transposed to [c_in, c_out] for the PE's lhsT layout
  - LN1/LN2 scale+bias folded into q/k/v and fc1 weights+biases
  - v weights+bias pre-scaled by SV so the fp8 eviction needs no extra op
  - final feature-LN scale/bias folded into the classifier head
  - weights cast to bf16; patch embed + head ride bf16 too

Schedule (vs the previous revision):
  - qk projection fused into the attention pipeline per output-column
    tile mc: scores for head pair mc follow qkproj(mc+1), so the ACT
    exp stream starts early and stays hidden behind PE work.
  - score matmuls row-packed: both heads of a pair run concurrently in
    disjoint PE row groups (K=64 each), into the two banks of a
    [128,1024] psum tile; one ACT exp covers both heads' j-chunk.
  - exp output is fp8(e4m3); AV runs fp8 DoubleRow (K=256/pass) with a
    ones-column in V producing the softmax denominator; reciprocal
    batched per head pair; normalization fused into the oT eviction.
  - fc2 (and patch) iterate t in (3,0,1,2) so the last token tile's
    LN chain overlaps the other tiles' matmuls; next layer's aT
    transposes then run stall-free.
  - final head weights prefetched in 2 big DMAs; the feature transpose
    runs inline with fc2 of layer 11; a dummy sqrt warms the ACT table.
"""

import os
import sys

import numpy as np

for _p in ("/opt/trn_rl_repo", "/root/.axon_site/_ro/trn_rl_repo"):
    if _p not in sys.path and os.path.isdir(_p):
        sys.path.insert(0, _p)

import ml_dtypes  # noqa: E402

import concourse.bass as bass  # noqa: E402,F401
import concourse.mybir as mybir  # noqa: E402
import concourse.tile as tile  # noqa: E402
from concourse import bacc  # noqa: E402
from concourse.bass_utils import run_bass_kernel_spmd  # noqa: E402
from concourse.masks import make_identity  # noqa: E402

F32 = mybir.dt.float32
BF16 = mybir.dt.bfloat16
FP8 = mybir.dt.float8e4
AF = mybir.ActivationFunctionType
OP = mybir.AluOpType
AX = mybir.AxisListType
DR = mybir.MatmulPerfMode.DoubleRow

DEPTH, C, NH, HD = 12, 768, 12, 64
NTOK, PVEC = 512, 512
FF = 3072
NCLS = 100
TB, SP = 8, 64
FD = TB * C
SCALE = HD**-0.5
EPS = 1e-5
SV = 32.0  # fp8 scale on the v path (weights+bias pre-scaled host-side)
VSL = NH * (HD + 1) + 4  # 784: per-key-pair-slot v row, padded so 784%16==0

CB_PER_LAYER = 36  # qb 6 + kb 6 + f1b 24 columns
CB_W1 = DEPTH * CB_PER_LAYER
CB_HB = CB_W1 + 1
CB_COLS = CB_HB + 1

bf16 = ml_dtypes.bfloat16


def _build():
    nc = bacc.Bacc(None, target_bir_lowering=False, debug=False)

    d_xpt = nc.dram_tensor("xpt", [PVEC, NTOK], BF16, kind="ExternalInput")
    d_pos2 = nc.dram_tensor("pos2", [NTOK, C], F32, kind="ExternalInput")
    d_pwt = nc.dram_tensor("pwt", [PVEC, C], BF16, kind="ExternalInput")
    d_wq = nc.dram_tensor("wq", [DEPTH, C, C], BF16, kind="ExternalInput")
    d_wk = nc.dram_tensor("wk", [DEPTH, C, C], BF16, kind="ExternalInput")
    d_wv = nc.dram_tensor("wv", [DEPTH, C, C], BF16, kind="ExternalInput")
    d_wp = nc.dram_tensor("wp", [DEPTH, C, C], BF16, kind="ExternalInput")
    d_w1 = nc.dram_tensor("w1", [DEPTH, C, FF], BF16, kind="ExternalInput")
    d_w2 = nc.dram_tensor("w2", [DEPTH, FF, C], BF16, kind="ExternalInput")
    d_cb = nc.dram_tensor("cb", [128, CB_COLS], F32, kind="ExternalInput")
    d_rb = nc.dram_tensor("rb", [DEPTH, 3, C], BF16, kind="ExternalInput")
    d_hwt = nc.dram_tensor("hwt", [4, 128, 12 * NCLS], BF16, kind="ExternalInput")
    d_out = nc.dram_tensor("out", [NCLS], F32, kind="ExternalOutput")

    from contextlib import ExitStack

    with tile.TileContext(nc) as tc:
        with ExitStack() as ctx:
            ep = ctx.enter_context
            const = ep(tc.tile_pool(name="const", bufs=1))
            hpool = ep(tc.tile_pool(name="hpool", bufs=4))
            arow_p = ep(tc.tile_pool(name="arow", bufs=4))
            aT_p = ep(tc.tile_pool(name="atp", bufs=1))
            a2T_p = ep(tc.tile_pool(name="a2tp", bufs=6))
            qT_p = ep(tc.tile_pool(name="qtp", bufs=6))
            kT_p = ep(tc.tile_pool(name="ktp", bufs=6))
            vx_p = ep(tc.tile_pool(name="vxp", bufs=2))
            ex_p = ep(tc.tile_pool(name="exp", bufs=3))
            oT_p = ep(tc.tile_pool(name="otp", bufs=6))
            gT_p = ep(tc.tile_pool(name="gtp", bufs=24))
            qkw_p = ep(tc.tile_pool(name="qkw", bufs=12))
            vpw_p = ep(tc.tile_pool(name="vpw", bufs=9))
            w1_p = ep(tc.tile_pool(name="w1p", bufs=12))
            w2_p = ep(tc.tile_pool(name="w2p", bufs=9))
            hw_p = ep(tc.tile_pool(name="hwp", bufs=2))
            bc_p = ep(tc.tile_pool(name="bcp", bufs=2))
            rcp_p = ep(tc.tile_pool(name="rcpp", bufs=2))
            den_p = ep(tc.tile_pool(name="denp", bufs=1))
            sm_p = ep(tc.tile_pool(name="smp", bufs=8))
            sm512_p = ep(tc.tile_pool(name="sm512", bufs=2))
            mm_ps = ep(tc.tile_pool(name="mmps", bufs=4, space="PSUM"))
            sc_ps = ep(tc.tile_pool(name="scps", bufs=2, space="PSUM"))

            ident = const.tile([128, 128], F32, tag="ident", name="ident")
            make_identity(nc, ident)
            identB = const.tile([128, 128], BF16, tag="identB", name="identB")
            nc.scalar.copy(identB[:], ident[:])
            ones0 = const.tile([128, 1], F32, tag="ones0", name="ones0")
            nc.vector.memset(ones0[:], 1.0)
            onesB = const.tile([128, 1], BF16, tag="onesB", name="onesB")
            nc.scalar.copy(onesB[:], ones0[:])
            ones_r = const.tile([1, 64], F32, tag="ones_r", name="ones_r")
            nc.vector.memset(ones_r[:], 1.0)
            eps = const.tile([128, 1], F32, tag="eps", name="eps")
            nc.vector.memset(eps[:], EPS)
            cb = const.tile([128, CB_COLS], F32, tag="cb", name="cb")
            nc.sync.dma_start(out=cb[:], in_=d_cb[:])

            h = []
            for t in range(4):
                ht = hpool.tile([128, C], F32, tag="h", name=f"h{t}")
                h.append(ht)

            def emit_stats0(t, tag):
                st6 = sm_p.tile([128, 12], F32, tag="st6", name=f"st6_{tag}{t}")
                nc.vector.bn_stats(st6[:, 0:6], h[t][:, 0:384])
                return st6

            def emit_ln_rest(t, tag, st6):
                nc.vector.bn_stats(st6[:, 6:12], h[t][:, 384:768])
                mv = sm_p.tile([128, 2], F32, tag="mv", name=f"mv{tag}{t}")
                nc.vector.bn_aggr(mv[:], st6.rearrange("p (g s) -> p g s", g=2))
                std = sm_p.tile([128, 1], F32, tag="std", name=f"std{tag}{t}")
                nc.scalar.activation(std[:], mv[:, 1:2], AF.Sqrt, bias=eps[:])
                rstd = sm_p.tile([128, 1], F32, tag="rstd", name=f"rstd{tag}{t}")
                nc.vector.reciprocal_approx_fast(out=rstd[:], in_=std[:])
                at = arow_p.tile([128, C], BF16, tag="ar", name=f"ar{tag}{t}")
                nc.vector.tensor_scalar(
                    at[:], h[t], mv[:, 0:1], rstd[:], op0=OP.subtract, op1=OP.mult
                )
                return at

            def transpose_pass(rows, t_list, col0, outs, tag2):
                """Transpose token tiles t_list into cols [col0:] of the 6
                col tiles; evictions alternate ACT/DVE on tp-tile halves."""
                w = 128 * len(t_list)
                tp = tp_ps.tile([128, 1024], BF16, tag="tp", name=f"tp{tag2}")
                for cc in range(6):
                    sl = tp[:, (cc % 2) * 512 : (cc % 2) * 512 + 512]
                    for ti, t in enumerate(t_list):
                        nc.tensor.transpose(
                            sl[:, ti * 128 : (ti + 1) * 128],
                            rows[t][:, cc * 128 : (cc + 1) * 128],
                            identB[:],
                        )
                    dst = outs[cc][:, col0 : col0 + w]
                    if cc % 2 == 0:
                        nc.scalar.copy(dst, sl[:, 0:w])
                    else:
                        nc.vector.tensor_copy(dst, sl[:, 0:w])

            def transpose_big(rows, order, big, tag2):
                """Per token tile: 6 PE transposes + 2 strided evictions into
                the [128, 6*512] column-major tile; tp halves double-buffer."""
                bigr = big.rearrange("p (c w) -> p c w", c=6)
                tp = tp_ps.tile([128, 1024], BF16, tag="tp", name=f"tp{tag2}")
                k = 0
                for t in order:
                    for c0, ncc in ((0, 4), (4, 2)):
                        sl = tp[:, (k % 2) * 512 : (k % 2) * 512 + 128 * ncc]
                        for ci in range(ncc):
                            nc.tensor.transpose(
                                sl[:, ci * 128 : (ci + 1) * 128],
                                rows[t][:, (c0 + ci) * 128 : (c0 + ci + 1) * 128],
                                identB[:],
                            )
                        dst = bigr[:, c0 : c0 + ncc, t * 128 : (t + 1) * 128]
                        srcv = sl.rearrange("p (c w) -> p c w", c=ncc)
                        if k % 2 == 0:
                            nc.scalar.copy(dst, srcv)
                        else:
                            nc.vector.tensor_copy(dst, srcv)
                        k += 1

            def bcast_row(i, j, tag):
                src = sm512_p.tile([1, C], BF16, tag="rbs", name=f"rbs{i}_{j}", bufs=1)
                nc.sync.dma_start(out=src[:], in_=d_rb[i, j])
                bt = bc_p.tile([128, C], BF16, tag="bc", name=f"{tag}{i}")
                nc.gpsimd.partition_broadcast(bt[:], src[:])
                return bt

            tp_k = [0]

            def transpose_one(row_t, t, big, tag2, eng="alt"):
                """6 PE transposes of one token tile into the [128, 6*512]
                column-major tile; 2 strided evictions (ACT, DVE, or both)."""
                bigr = big.rearrange("p (c w) -> p c w", c=6)
                for c0, ncc in ((0, 4), (4, 2)):
                    k = tp_k[0]
                    tp_k[0] += 1
                    sl = mm_ps.tile(
                        [128, 128 * ncc], BF16, tag="mm", name=f"tq{tag2}{t}_{c0}"
                    )
                    for ci in range(ncc):
                        nc.tensor.transpose(
                            sl[:, ci * 128 : (ci + 1) * 128],
                            row_t[:, (c0 + ci) * 128 : (c0 + ci + 1) * 128],
                            identB[:],
                        )
                    dst = bigr[:, c0 : c0 + ncc, t * 128 : (t + 1) * 128]
                    srcv = sl.rearrange("p (c w) -> p c w", c=ncc)
                    on_act = (eng == "act") or (eng == "alt" and k % 2 == 0)
                    if on_act:
                        nc.scalar.copy(dst, srcv)
                    else:
                        nc.vector.tensor_copy(dst, srcv)

            # ---- patch embed: h = pos(+patch_b) + xp @ patch_w.T ----
            # lag-1 LN + transposes into layer 0's aT; t3 left pending.
            xpt = []
            pwt = []
            for kc in range(4):
                xt = qT_p.tile([128, NTOK], BF16, tag="qt", name=f"xpt{kc}")
                nc.sync.dma_start(out=xt[:], in_=d_xpt[kc * 128 : (kc + 1) * 128, :])
                xpt.append(xt)
            for kc in range(4):
                wt = vpw_p.tile([128, C], BF16, tag="vpw", name=f"pwt{kc}")
                nc.sync.dma_start(out=wt[:], in_=d_pwt[kc * 128 : (kc + 1) * 128, :])
                pwt.append(wt)
            a_rows = [None] * 4
            aT = aT_p.tile([128, 6 * NTOK], BF16, tag="at", name="at_l0", bufs=1)
            for t in range(4):
                nc.sync.dma_start(out=h[t][:], in_=d_pos2[t * 128 : (t + 1) * 128, :])
                st6 = None
                for n in range(2):
                    ns = slice(n * 384, (n + 1) * 384)
                    ps = mm_ps.tile([128, 512], F32, tag="mm", name=f"pep{t}{n}")
                    for kc in range(4):
                        nc.tensor.matmul(
                            ps[:, :384],
                            xpt[kc][:, t * 128 : (t + 1) * 128],
                            pwt[kc][:, ns],
                            start=(kc == 0),
                            stop=(kc == 3),
                        )
                    nc.vector.tensor_tensor(h[t][:, ns], h[t][:, ns], ps[:, :384], op=OP.add)
                    if n == 0:
                        st6 = emit_stats0(t, "a0_")
                a_rows[t] = emit_ln_rest(t, "a0_", st6)
                if t >= 1:
                    transpose_one(a_rows[t - 1], t - 1, aT, "pa")
            transpose_one(a_rows[2], 2, aT, "pb")
            # a_rows[3] transpose pending; done at layer-0 boundary

            for i in range(DEPTH):
                cb0 = i * CB_PER_LAYER
                # ---- weights for this layer ----
                qk_w = []
                for (dw, tg) in ((d_wq, "qw"), (d_wk, "kw")):
                    wts = []
                    for kc in range(6):
                        wt = qkw_p.tile([128, C], BF16, tag="qkw", name=f"{tg}{kc}")
                        nc.sync.dma_start(out=wt[:], in_=dw[i, kc * 128 : (kc + 1) * 128, :])
                        wts.append(wt)
                    qk_w.append(wts)
                vwts = []
                for kc in range(6):
                    wt = vpw_p.tile([128, C], BF16, tag="vpw", name=f"vw{kc}")
                    nc.sync.dma_start(out=wt[:], in_=d_wv[i, kc * 128 : (kc + 1) * 128, :])
                    vwts.append(wt)
                pwts = []
                for kc in range(6):
                    wt = vpw_p.tile([128, C], BF16, tag="vpw", name=f"pw{kc}")
                    nc.sync.dma_start(out=wt[:], in_=d_wp[i, kc * 128 : (kc + 1) * 128, :])
                    pwts.append(wt)

                vbB = bcast_row(i, 0, "vb")
                pbB = bcast_row(i, 1, "pb")

                vx = []
                for g in range(2):
                    vt = vx_p.tile([128, 2 * VSL], FP8, tag="vx", name=f"vx{g}")
                    vx.append(vt)
                    for s in range(2):
                        ones_sl = vt[:, s * VSL : s * VSL + NH * 65].rearrange(
                            "p (h d) -> p h d", h=NH
                        )[:, :, HD : HD + 1]
                        nc.vector.memset(ones_sl, 1.0)

                qT = [None] * 6
                kT = [None] * 6
                ex_all = [None] * 6
                oT = []
                for cc in range(6):
                    ot = oT_p.tile([128, NTOK], BF16, tag="ot", name=f"ot{cc}")
                    oT.append(ot)

                def qkproj0a(which):
                    wts, base = (qk_w[0], cb0) if which == 0 else (qk_w[1], cb0 + 6)
                    out = (qT_p if which == 0 else kT_p).tile(
                        [128, NTOK], BF16, tag="qt" if which == 0 else "kt",
                        name=f"{'qk'[which]}T0",
                    )
                    ps = mm_ps.tile([128, 512], F32, tag="mm", name=f"qk0a_{which}")
                    for k in range(6):
                        nc.tensor.matmul(
                            ps[:, 0:384],
                            wts[k][:, 0:128],
                            aT[:, k * 512 : k * 512 + 384],
                            start=(k == 0),
                            stop=(k == 5),
                        )
                    if which == 0:
                        nc.scalar.activation(
                            out[:, 0:384], ps[:, 0:384], AF.Identity, bias=cb[:, base : base + 1]
                        )
                    else:
                        nc.vector.tensor_scalar_add(
                            out[:, 0:384], ps[:, 0:384], cb[:, base : base + 1]
                        )
                    return out

                def qkproj0b(which, out):
                    wts, base = (qk_w[0], cb0) if which == 0 else (qk_w[1], cb0 + 6)
                    ps = mm_ps.tile([128, 512], F32, tag="mm", name=f"qk0b_{which}")
                    for k in range(6):
                        nc.tensor.matmul(
                            ps[:, 0:128],
                            wts[k][:, 0:128],
                            aT[:, k * 512 + 384 : k * 512 + 512],
                            start=(k == 0),
                            stop=(k == 5),
                        )
                    if which == 0:
                        nc.scalar.activation(
                            out[:, 384:512], ps[:, 0:128], AF.Identity, bias=cb[:, base : base + 1]
                        )
                    else:
                        nc.vector.tensor_scalar_add(
                            out[:, 384:512], ps[:, 0:128], cb[:, base : base + 1]
                        )

                def emit_qkproj_one(mc, which):
                    wts, outs, base = (
                        (qk_w[0], qT, cb0) if which == 0 else (qk_w[1], kT, cb0 + 6)
                    )
                    ps = mm_ps.tile([128, 512], F32, tag="mm", name=f"qkp{mc}_{which}")
                    for k in range(6):
                        kc = (k + mc) % 6
                        nc.tensor.matmul(
                            ps[:],
                            wts[kc][:, mc * 128 : (mc + 1) * 128],
                            aT[:, kc * 512 : (kc + 1) * 512],
                            start=(k == 0),
                            stop=(k == 5),
                        )
                    out = (qT_p if which == 0 else kT_p).tile(
                        [128, NTOK], BF16, tag="qt" if which == 0 else "kt",
                        name=f"{'qk'[which]}T{mc}",
                    )
                    if which == 0:
                        nc.scalar.activation(
                            out[:], ps[:], AF.Identity, bias=cb[:, base + mc : base + mc + 1]
                        )
                    else:
                        nc.vector.tensor_scalar_add(
                            out[:], ps[:], cb[:, base + mc : base + mc + 1]
                        )
                    outs[mc] = out

                def emit_pair_scores(p, jlist, expair):
                    for j in jlist:
                        sc = sc_ps.tile([128, 1024], F32, tag="sc", name=f"sc{p}_{j}")
                        for hi in range(2):
                            off = hi * 64
                            nc.tensor.matmul(
                                sc[:, hi * 512 : (hi + 1) * 512],
                                kT[p][off : off + 64, j * 128 : (j + 1) * 128],
                                qT[p][off : off + 64, :],
                                start=True,
                                stop=True,
                            )
                        nc.scalar.activation(
                            expair[:, j * 1024 : (j + 1) * 1024], sc[:], AF.Exp,
                            scale=SCALE,
                        )
                    ex_all[p] = expair

                def emit_vgroup(t):
                    vxt = vx[t // 2]
                    base = (t % 2) * VSL
                    for n in range(2):
                        ps = mm_ps.tile([128, 512], F32, tag="mm", name=f"vp{t}{n}")
                        for kc in range(6):
                            nc.tensor.matmul(
                                ps[:, :384],
                                aT[:, kc * 512 + t * 128 : kc * 512 + (t + 1) * 128],
                                vwts[kc][:, n * 384 : (n + 1) * 384],
                                start=(kc == 0),
                                stop=(kc == 5),
                            )
                        dst = vxt[:, base + n * 6 * 65 : base + (n + 1) * 6 * 65].rearrange(
                            "p (h d) -> p h d", h=6
                        )[:, :, 0:HD]
                        nc.vector.tensor_tensor(
                            dst,
                            ps[:, :384].rearrange("p (g d) -> p g d", g=6),
                            vbB[:, n * 384 : (n + 1) * 384].rearrange(
                                "p (g d) -> p g d", g=6
                            ),
                            op=OP.add,
                        )

                def emit_pair_av(p):
                    exr = ex_all[p].rearrange("q (j c) -> q j c", j=4)
                    pos_ = []
                    rcp2 = den_p.tile([1, 2 * NTOK], F32, tag="dr", name=f"rcp{p}", bufs=1)
                    for hi in range(2):
                        hh = 2 * p + hi
                        po = mm_ps.tile([128, 512], F32, tag="mm", name=f"po{hh}")
                        for jp in range(2):
                            nc.tensor.matmul(
                                po[0 : HD + 1, :],
                                vx[jp].rearrange("q (s c) -> q s c", s=2)[
                                    :, :, hh * 65 : hh * 65 + 65
                                ],
                                exr[:, 2 * jp : 2 * jp + 2, hi * 512 : (hi + 1) * 512],
                                start=(jp == 0),
                                stop=(jp == 1),
                                perf_mode=DR,
                            )
                        den = den_p.tile([1, NTOK], F32, tag="dn", name=f"den{hh}", bufs=1)
                        nc.vector.tensor_scalar_mul(den[:], po[HD : HD + 1, :], SV)
                        nc.vector.reciprocal_approx_fast(
                            out=rcp2[:, hi * NTOK : (hi + 1) * NTOK], in_=den[:]
                        )
                        pos_.append(po)
                    rb_ = rcp_p.tile([64, 2 * NTOK], F32, tag="rb", name=f"rcpB{p}", bufs=1)
                    nc.gpsimd.partition_broadcast(rb_[:], rcp2[:])
                    for hi in range(2):
                        hh = 2 * p + hi
                        nc.vector.tensor_tensor(
                            oT[hh // 2][(hh % 2) * 64 : (hh % 2) * 64 + 64, :],
                            pos_[hi][0:HD, :],
                            rb_[:, hi * NTOK : (hi + 1) * NTOK],
                            op=OP.mult,
                        )

                # ---- boundary: split qk-proj for mc=0 around the pending
                # t3 transpose, then the mc pipeline ----
                out_q0 = qkproj0a(0)
                out_k0 = qkproj0a(1)
                transpose_one(a_rows[3], 3, aT, f"bd{i}")
                qkproj0b(0, out_q0)
                qkproj0b(1, out_k0)
                qT[0] = out_q0
                kT[0] = out_k0

                for mc in range(1, 6):
                    emit_qkproj_one(mc, 0)
                    ex_t = ex_p.tile([128, 4096], FP8, tag="ex", name=f"ex{mc - 1}")
                    emit_pair_scores(mc - 1, (0, 1), ex_t)
                    emit_qkproj_one(mc, 1)
                    emit_pair_scores(mc - 1, (2, 3), ex_t)
                    if mc == 1:
                        emit_vgroup(0)
                        emit_vgroup(1)
                    if mc == 2:
                        emit_vgroup(2)
                        emit_vgroup(3)
                    if mc == 3:
                        emit_pair_av(0)
                    if mc == 4:
                        emit_pair_av(1)
                        emit_pair_av(2)
                    if mc == 5:
                        emit_pair_av(3)
                ex_t = ex_p.tile([128, 4096], FP8, tag="ex", name="ex5")
                emit_pair_scores(5, (0, 1), ex_t)
                emit_pair_scores(5, (2, 3), ex_t)
                emit_pair_av(4)
                emit_pair_av(5)

                # ---- output projection + residual (pb pre-added), lag-1 LN2
                # transposes into a2T; t3 pending into fc1 ----
                a2T = a2T_p.tile([128, 6 * NTOK], BF16, tag="a2t", name=f"a2t{i}", bufs=1)
                a2_rows = [None] * 4
                for t in range(4):
                    nc.vector.tensor_tensor(h[t][:], h[t][:], pbB[:], op=OP.add)
                    st6 = None
                    for n in range(2):
                        ns = slice(n * 384, (n + 1) * 384)
                        ps = mm_ps.tile([128, 512], F32, tag="mm", name=f"prj{t}{n}")
                        for k in range(6):
                            nc.tensor.matmul(
                                ps[:, :384],
                                oT[k][:, t * 128 : (t + 1) * 128],
                                pwts[k][:, ns],
                                start=(k == 0),
                                stop=(k == 5),
                            )
                        nc.vector.tensor_tensor(h[t][:, ns], h[t][:, ns], ps[:, :384], op=OP.add)
                        if n == 0:
                            st6 = emit_stats0(t, f"b{i}_")
                    a2_rows[t] = emit_ln_rest(t, f"b{i}_", st6)
                    if t >= 1:
                        transpose_one(a2_rows[t - 1], t - 1, a2T, f"p{i}_", eng="act")
                transpose_one(a2_rows[2], 2, a2T, f"p{i}b_", eng="act")

                # ---- fc1 + gelu; the 0:384 pass hides t3's LN + transpose ----
                f2bB = bcast_row(i, 2, "fb")
                gT = [gT_p.tile([128, NTOK], BF16, tag="gt", name=f"gt{m}") for m in range(24)]
                for half in range(2):
                    wts = []
                    for kc in range(6):
                        wt = w1_p.tile([128, FF // 2], BF16, tag="w1", name=f"w1_{half}_{kc}")
                        nc.sync.dma_start(
                            out=wt[:],
                            in_=d_w1[
                                i,
                                kc * 128 : (kc + 1) * 128,
                                half * (FF // 2) : (half + 1) * (FF // 2),
                            ],
                        )
                        wts.append(wt)
                    def f1_mm(m, mh, cs, ce):
                        ps = mm_ps.tile([128, 512], F32, tag="mm", name=f"f1p{m}_{cs}")
                        for k in range(6):
                            kc = (k + mh) % 6
                            nc.tensor.matmul(
                                ps[:, 0 : ce - cs],
                                wts[kc][:, mh * 128 : (mh + 1) * 128],
                                a2T[:, kc * 512 + cs : kc * 512 + ce],
                                start=(k == 0),
                                stop=(k == 5),
                            )
                        nc.scalar.activation(
                            gT[m][:, cs:ce], ps[:, 0 : ce - cs], AF.Gelu,
                            bias=cb[:, cb0 + 12 + m : cb0 + 13 + m],
                        )

                    if half == 0:
                        for mh in range(3):
                            f1_mm(mh, mh, 0, 384)
                        transpose_one(a2_rows[3], 3, a2T, f"p{i}c_", eng="act")
                        for mh in range(3):
                            f1_mm(mh, mh, 384, 512)
                        for t in range(4):
                            nc.vector.tensor_tensor(h[t][:], h[t][:], f2bB[:], op=OP.add)
                        for mh in range(3, 12):
                            f1_mm(mh, mh, 0, 512)
                    else:
                        for mh in range(12):
                            f1_mm(12 + mh, mh, 0, 512)

                # ---- fc2 + residual; lag-1 next-layer LN1 + aT transposes
                # (or the final feature transpose on the last layer) ----
                last = i == DEPTH - 1
                if last:
                    hw = []
                    for g in range(4):
                        hwt_t = hw_p.tile([128, 12 * NCLS], BF16, tag="hw", name=f"hw{g}")
                        nc.sync.dma_start(out=hwt_t[:], in_=d_hwt[g])
                        hw.append(hwt_t)
                    dum = sm_p.tile([1, 1], F32, tag="dum", name="dum")
                    nc.scalar.activation(dum[:], eps[0:1, :], AF.Sqrt)
                    hTa = w1_p.tile([128, 3 * NTOK], BF16, tag="w1", name="hTa")
                    hTb = w1_p.tile([128, 3 * NTOK], BF16, tag="w1", name="hTb")

                    def hTr(cc):
                        t_ = hTa if cc < 3 else hTb
                        return t_.rearrange("p (c w) -> p c w", c=3)[:, cc % 3, :]

                    def emit_ftr(t, hbt):
                        for c0, dtile in ((0, hTa), (3, hTb)):
                            k = tp_k[0]
                            tp_k[0] += 1
                            sl = mm_ps.tile(
                                [128, 384], BF16, tag="mm", name=f"tf{t}_{c0}"
                            )
                            for ci in range(3):
                                nc.tensor.transpose(
                                    sl[:, ci * 128 : (ci + 1) * 128],
                                    hbt[:, (c0 + ci) * 128 : (c0 + ci + 1) * 128],
                                    identB[:],
                                )
                            dst = dtile.rearrange("p (c w) -> p c w", c=3)[
                                :, :, t * 128 : (t + 1) * 128
                            ]
                            srcv = sl.rearrange("p (c w) -> p c w", c=3)
                            if k % 2 == 0:
                                nc.scalar.copy(dst, srcv)
                            else:
                                nc.vector.tensor_copy(dst, srcv)
                else:
                    aT_next = aT_p.tile(
                        [128, 6 * NTOK], BF16, tag="at", name=f"at{i + 1}", bufs=1
                    )
                new_rows = [None] * 4
                st6s = [None] * 4
                tdone = []
                for n in range(2):
                    ns = slice(n * 384, (n + 1) * 384)
                    w2ts = []
                    for jq in range(6):
                        wt = w2_p.tile([128, 4 * 384], BF16, tag="w2", name=f"w2_{n}_{jq}")
                        nc.sync.dma_start(
                            out=wt.rearrange("p (c w) -> p c w", c=4),
                            in_=d_w2[i, jq * 512 : (jq + 1) * 512, ns].rearrange(
                                "(c p) w -> p c w", c=4
                            ),
                        )
                        w2ts.append(wt)
                    for t in range(4):
                        ps = mm_ps.tile([128, 512], F32, tag="mm", name=f"f2p{t}{n}")
                        for jc in range(24):
                            nc.tensor.matmul(
                                ps[:, :384],
                                gT[jc][:, t * 128 : (t + 1) * 128],
                                w2ts[jc // 4][:, (jc % 4) * 384 : (jc % 4 + 1) * 384],
                                start=(jc == 0),
                                stop=(jc == 23),
                            )
                        nc.vector.tensor_tensor(h[t][:, ns], h[t][:, ns], ps[:, :384], op=OP.add)
                        if n == 0:
                            st6s[t] = emit_stats0(t, f"a{i + 1}_")
                        elif not last:
                            new_rows[t] = emit_ln_rest(t, f"a{i + 1}_", st6s[t])
                            if t >= 1:
                                transpose_one(new_rows[t - 1], t - 1, aT_next, f"f{i}_", eng="act")
                        else:
                            hbt = arow_p.tile([128, C], BF16, tag="ar", name=f"hb{t}")
                            nc.scalar.copy(hbt[:], h[t][:])
                            tdone.append((t, hbt))
                            if len(tdone) >= 2:
                                emit_ftr(*tdone[-2])
                if last:
                    emit_ftr(*tdone[-1])
                else:
                    transpose_one(new_rows[2], 2, aT_next, f"f{i}b_", eng="act")
                    a_rows = new_rows
                    aT = aT_next

            # ---- final: feature-LN stats + head (hT built inline above) ----
            ps_s = mm_ps.tile([128, 512], F32, tag="mm", name="ps_s")
            ps_q = mm_ps.tile([128, 512], F32, tag="mm", name="ps_q")
            for cc in range(6):
                s = aT_p.tile([128, NTOK], BF16, tag="at", name=f"sq{cc}")
                nc.scalar.activation(s[:], hTr(cc), AF.Square)
                nc.tensor.matmul(
                    ps_s[0:1, :], onesB[:], hTr(cc), start=(cc == 0), stop=(cc == 5)
                )
                nc.tensor.matmul(
                    ps_q[0:1, :], onesB[:], s[:], start=(cc == 0), stop=(cc == 5)
                )
            sum_s = sm512_p.tile([1, SP], F32, tag="rbs", name="sum_s", bufs=1)
            nc.vector.tensor_reduce(
                sum_s[:], ps_s[0:1, :].rearrange("p (g s) -> p s g", g=TB),
                axis=AX.X, op=OP.add,
            )
            sum_q = sm512_p.tile([1, SP], F32, tag="rbs", name="sum_q", bufs=1)
            nc.vector.tensor_reduce(
                sum_q[:], ps_q[0:1, :].rearrange("p (g s) -> p s g", g=TB),
                axis=AX.X, op=OP.add,
            )
            mean = sm512_p.tile([1, SP], F32, tag="mn", name="mean")
            nc.vector.tensor_scalar_mul(mean[:], sum_s[:], 1.0 / FD)
            msq = sm512_p.tile([1, SP], F32, tag="mn", name="msq")
            nc.vector.tensor_scalar_mul(msq[:], sum_q[:], 1.0 / FD)
            mm2 = sm512_p.tile([1, SP], F32, tag="rcp", name="mm2")
            nc.vector.tensor_tensor(mm2[:], mean[:], mean[:], op=OP.mult)
            var = sm512_p.tile([1, SP], F32, tag="rcp", name="var")
            nc.vector.tensor_tensor(var[:], msq[:], mm2[:], op=OP.subtract)
            stdf = sm512_p.tile([1, SP], F32, tag="rcp", name="stdf")
            nc.scalar.activation(stdf[:], var[:], AF.Sqrt, bias=eps[0:1, :])
            rstd = sm512_p.tile([1, SP], F32, tag="rcp", name="rstdf")
            nc.vector.reciprocal_approx_fast(out=rstd[:], in_=stdf[:])
            rstdB = sm_p.tile([128, SP], F32, tag="rstdB", name="rstdB", bufs=1)
            nc.gpsimd.partition_broadcast(rstdB[:, 0:SP], rstd[:])
            cm = sm512_p.tile([1, SP], F32, tag="rcp", name="cm")
            nc.vector.tensor_tensor(cm[:], mean[:], rstd[:], op=OP.mult)
            c0 = sm512_p.tile([1, 1], F32, tag="c0", name="c0")
            nc.vector.tensor_reduce(c0[:], cm[:], axis=AX.X, op=OP.add)
            c0B = sm_p.tile([128, 1], F32, tag="c0b", name="c0B")
            nc.gpsimd.partition_broadcast(c0B[:], c0[:])

            ps_l = mm_ps.tile([128, 512], F32, tag="mm", name="ps_l")
            for idx in range(48):
                cc, tb = idx // TB, idx % TB
                g, c = idx // 12, idx % 12
                nc.tensor.matmul(
                    ps_l[0:NCLS, 0:SP],
                    hw[g][:, c * NCLS : (c + 1) * NCLS],
                    hTr(cc)[:, tb * SP : (tb + 1) * SP],
                    start=(idx == 0),
                    stop=(idx == 47),
                )
            gs = sm_p.tile([128, SP], F32, tag="gs", name="gs", bufs=1)
            nc.vector.tensor_tensor(gs[0:NCLS, :], ps_l[0:NCLS, 0:SP], rstdB[0:NCLS, :], op=OP.mult)
            red = sm_p.tile([128, 1], F32, tag="red", name="red", bufs=1)
            nc.vector.tensor_reduce(red[0:NCLS, :], gs[0:NCLS, :], axis=AX.X, op=OP.add)
            t1 = sm_p.tile([128, 1], F32, tag="t1", name="t1")
            nc.vector.tensor_scalar(
                t1[0:NCLS, :],
                cb[0:NCLS, CB_W1 : CB_W1 + 1],
                c0B[0:NCLS, :],
                None,
                op0=OP.mult,
            )
            t2 = sm_p.tile([128, 1], F32, tag="t2", name="t2")
            nc.vector.tensor_tensor(t2[0:NCLS, :], red[0:NCLS, :], t1[0:NCLS, :], op=OP.subtract)
            logits = sm_p.tile([128, 1], F32, tag="lg", name="logits")
            nc.vector.tensor_scalar(
                logits[0:NCLS, :],
                t2[0:NCLS, :],
                1.0 / SP,
                cb[0:NCLS, CB_HB : CB_HB + 1],
                op0=OP.mult,
                op1=OP.add,
            )
            nc.sync.dma_start(out=d_out[:], in_=logits[0:NCLS, :])

    nc.compile()
    return nc


def _prep_inputs(inputs):
    f = np.float32
    x = np.asarray(inputs["x"], f)
    B = x.shape[0]
    xpt = np.empty((B, PVEC, NTOK), bf16)
    for b in range(B):
        xp = x[b, 0].reshape(8, 8, 8, 8, 8, 8).transpose(0, 2, 4, 1, 3, 5).reshape(NTOK, PVEC)
        xpt[b] = np.ascontiguousarray(xp.T).astype(bf16)

    qw, kw, vw, pw = (np.asarray(inputs[k], f) for k in ("qw", "kw", "vw", "pw"))
    f1w, f2w = np.asarray(inputs["f1w"], f), np.asarray(inputs["f2w"], f)
    l1w, l1b = np.asarray(inputs["ln1_w"], f), np.asarray(inputs["ln1_b"], f)
    l2w, l2b = np.asarray(inputs["ln2_w"], f), np.asarray(inputs["ln2_b"], f)

    wq = np.ascontiguousarray((qw * l1w[:, None, :]).transpose(0, 2, 1)).astype(bf16)
    wk = np.ascontiguousarray((kw * l1w[:, None, :]).transpose(0, 2, 1)).astype(bf16)
    wv = np.ascontiguousarray(
        (vw * l1w[:, None, :] * SV).transpose(0, 2, 1)
    ).astype(bf16)
    wp = np.ascontiguousarray(pw.transpose(0, 2, 1)).astype(bf16)
    w1 = np.ascontiguousarray((f1w * l2w[:, None, :]).transpose(0, 2, 1)).astype(bf16)
    w2 = np.ascontiguousarray(f2w.transpose(0, 2, 1)).astype(bf16)

    qb = np.asarray(inputs["qb"], f) + np.einsum("ioc,ic->io", qw, l1b)
    kb = np.asarray(inputs["kb"], f) + np.einsum("ioc,ic->io", kw, l1b)
    vb = (np.asarray(inputs["vb"], f) + np.einsum("ioc,ic->io", vw, l1b)) * SV
    f1b = np.asarray(inputs["f1b"], f) + np.einsum("ijc,ic->ij", f1w, l2b)

    head_w = np.asarray(inputs["head_w"], f)
    fcn_w, fcn_b = np.asarray(inputs["fcn_w"], f), np.asarray(inputs["fcn_b"], f)
    head_b = np.asarray(inputs["head_b"], f) + head_w @ fcn_b
    hwt = np.ascontiguousarray(head_w.T * fcn_w[:, None])
    hwt_b = hwt.astype(bf16)
    # pack the 48 [128, 100] contraction chunks 24-per-tile in (cc, tb)
    # consumption order so the device loads 2 big tiles
    hwt_pk = np.zeros((4, 128, 12 * NCLS), bf16)
    for idx in range(48):
        cc, tb = idx // TB, idx % TB
        row0 = tb * C + cc * 128
        g, c = idx // 12, idx % 12
        hwt_pk[g, :, c * NCLS : (c + 1) * NCLS] = hwt_b[row0 : row0 + 128, :]

    cbp = np.zeros((128, CB_COLS), f)
    for i in range(DEPTH):
        c0 = i * CB_PER_LAYER
        cbp[:, c0 : c0 + 6] = qb[i].reshape(6, 128).T
        cbp[:, c0 + 6 : c0 + 12] = kb[i].reshape(6, 128).T
        cbp[:, c0 + 12 : c0 + 36] = f1b[i].reshape(24, 128).T
    cbp[:NCLS, CB_W1] = hwt_b.astype(f).sum(axis=0)
    cbp[:NCLS, CB_HB] = head_b

    rb = np.stack(
        [
            np.stack(
                [vb[i], np.asarray(inputs["pb"], f)[i], np.asarray(inputs["f2b"], f)[i]]
            )
            for i in range(DEPTH)
        ]
    ).astype(bf16)

    pos2 = (
        np.asarray(inputs["pos_embed"], f)[0] + np.asarray(inputs["patch_b"], f)[None, :]
    ).astype(f)
    pwt = np.ascontiguousarray(np.asarray(inputs["patch_w"], f).T).astype(bf16)

    shared = {
        "pos2": pos2,
        "pwt": pwt,
        "wq": wq,
        "wk": wk,
        "wv": wv,
        "wp": wp,
        "w1": w1,
        "w2": w2,
        "cb": cbp,
        "rb": rb,
        "hwt": hwt_pk,
    }
    return xpt, shared


_NC = None


def _get_nc():
    global _NC
    if _NC is None:
        _NC = _build()
    return _NC


def kernel(**inputs):
    nc = _get_nc()
    xpt, shared = _prep_inputs(inputs)
    B = xpt.shape[0]
    in_maps = [dict(shared, xpt=xpt[b]) for b in range(B)]
    res = run_bass_kernel_spmd(nc, in_maps, list(range(B)))
    return np.stack([res.results[b]["out"] for b in range(B)]).astype(np.float32)
